# revision 22
# baseline (speedup 1.0000x reference)
"""Trainium2 Bass kernel for a transformer encoder layer (nn_Encoder).

x:[2,2048,1024] f32. 8 NeuronCores, data-parallel: core c handles batch
n=c//4, query rows qi=c%4 (512 tokens). K/V are recomputed per core for
the full batch (x4 redundancy) to avoid collectives (~300us for the
8.4MB all-reduce this would replace).

All matmuls are fp8 e4m3 DoubleRow (2x PE column rate, 256-deep
contraction per instruction). K/Q are produced in a split-hd layout
([32 partitions, 2 k-tiles] per head) so even the hd=64 score matmuls
run DoubleRow. Weights are host-scaled x16 into the fp8 normal range;
the scale unwinds via the softmax ones-row (=16), the exp scale
(0.125/256) and a /256 on the fc2 PSUM. Residuals/LayerNorm stay f32.

The softmax exp stream on the Activation engine (~133us) is the
critical resource. Attention is split into two query-halves: while
half 1's exps run, half 0's output-proj/LN1/FFN execute in the PE/DVE
shadow (emitted as interleaved filler chunks); K/V projections for
later head-quads fill the shadow of half 0.
"""
import os
import sys
from contextlib import ExitStack

for _p in ("/opt/trn_rl_repo", "/root/.axon_site/_ro/trn_rl_repo"):
    if os.path.isdir(_p) and _p not in sys.path:
        sys.path.insert(0, _p)

import numpy as np
import concourse.bass as bass
import concourse.mybir as mybir
import concourse.tile as tile
from concourse import bacc
from concourse.bass_utils import run_bass_kernel_spmd
from concourse.masks import make_identity

F32 = mybir.dt.float32
F8 = mybir.dt.float8e4
AF = mybir.ActivationFunctionType
ALU = mybir.AluOpType
DR = mybir.MatmulPerfMode.DoubleRow

D = 1024
H = 16
HD = 64
FF = 4096
L = 2048
NB = 2
P = 128
QT = 512          # query tokens per core
QH = QT // 2      # query half
DC = D // P       # 8 chunks of the model dim
DC2 = DC // 2     # 4 DoubleRow chunk-pairs
KT = L // P       # 16 key tiles
FC = FF // P      # 32 ff chunks
FC2 = FC // 2     # 16 ff chunk-pairs
TT = QT // P      # 4 own-token tiles
NPAIR = H // 2    # 8 head pairs
NP2 = NPAIR // 2  # 4 pair-pairs
NQ = H // 4       # 4 head quads (scores split layout)
EPS = 1e-5
WS = 16.0         # host weight scale (fp8 range)

_CACHED_NC = {}


def _build_nc(affine=True):
    nc = bacc.Bacc("TRN2", target_bir_lowering=False)

    def dparam(name, shape, dt=F8):
        return nc.dram_tensor(name, shape, dt, kind="ExternalInput")

    xT8 = dparam("xT8", [D, L])            # x[n].T, fp8
    xTq8 = dparam("xTq8", [D, QT])         # own-token columns of xT, fp8
    xq = dparam("xq", [QT, D], F32)        # own tokens, natural (residual)
    # weights: partition-major fp8, DoubleRow k-tile-pair layouts
    wqs = dparam("wqs", [P, 2 * NQ, 2 * DC2 * P])    # [p][(g i)][d2 ik m]
    wks = dparam("wks", [P, 2 * NQ, 2 * DC2 * P])
    wv8 = dparam("wv8", [P, 2 * DC2 * 2 * 512])      # [p][vh d2 ik n]
    wo8 = dparam("wo8", [P, NP2 * 2 * D])            # [p][j ik o]
    w18 = dparam("w18", [P, FC // 4, 4 * (DC2 + 1) * 2 * P])  # [p][f4][f d2 ik m]
    w28 = dparam("w28", [P, FC2 // 2, 2 * 2 * D])        # [p][s][g ik o]
    bqs = dparam("bqs", [P, 2 * NQ], F32)  # x16 biases, split-hd order
    bks = dparam("bks", [P, 2 * NQ], F32)
    b1 = dparam("b1", [P, FC], F32)
    bvb = dparam("bvb", [P, D], F32)
    b2b = dparam("b2b", [P, D], F32)       # natural scale
    g1b = dparam("g1b", [P, D], F32)
    be1b = dparam("be1b", [P, D], F32)
    g2b = dparam("g2b", [P, D], F32)
    be2b = dparam("be2b", [P, D], F32)
    vones = dparam("vones", [P, KT])       # value 16 (denominator row)
    hc8 = dparam("hc8", [P, 2 * QT])       # F1 bias rows: [16,0...;0...]

    y = nc.dram_tensor("y", [QT, D], F32, kind="ExternalOutput")

    with tile.TileContext(nc) as tc, ExitStack() as es:
        # Pre-load the one activation table that serves every ACT func we
        # use (Exp for softmax, Ln+Exp for the LN rsqrt): without this the
        # first-fit chooser thrashes exp<->ln tables at 1283ns per load.
        from concourse.hw_specs import get_activation_tables
        _tabs = get_activation_tables(nc.m.arch)
        _idx = next(i for i, (_, s) in enumerate(_tabs.items())
                    if AF.Exp in s and AF.Ln in s)
        nc.scalar.add_instruction(mybir.InstLoadActFuncSet(
            name=nc.scalar.bass.get_next_instruction_name(),
            act_func_set_id=_idx, ins=[], outs=[]))

        pers = es.enter_context(tc.tile_pool(name="pers", bufs=1))
        ident = pers.tile([P, P], F32, tag="ident")
        make_identity(nc, ident[:])
        bqs_t = pers.tile([P, 2 * NQ], F32, tag="bqs")
        bks_t = pers.tile([P, 2 * NQ], F32, tag="bks")
        b1_t = pers.tile([P, FC], F32, tag="b1")
        bvb_t = pers.tile([P, D], F32, tag="bvb")
        # b2b reuses bvb's slot: bvb is dead after V-proj, long before
        # the LNT chunks fold b2 into the residual.
        b2b_t = pers.tile([P, D], F32, tag="bvb", name="b2b_t")

        xT_t = pers.tile([P, DC, L], F8, tag="xT")
        xTq_t = pers.tile([P, DC, QT], F8, tag="xTq")
        v_aug = pers.tile([P, KT, H * (HD + 1)], F8, tag="vaug")
        ones_t = pers.tile([P, KT], F8, tag="ones")
        qT_all = pers.tile([P, NQ, 2, QT], F8, tag="qT")
        kT_all = pers.tile([P, NQ, 2, L], F8, tag="kT")
        outSB = pers.tile([P, NPAIR, QT], F8, tag="outSB")
        h_t = pers.tile([P, TT, D], F32, tag="h")
        hT_t = pers.tile([P, DC + 2, QT], F8, tag="hT")
        ff1 = pers.tile([P, FC, QT], F8, tag="ff1")
        xq_s = pers.tile([P, TT, D], F32, tag="xqs")
        wqs_t = pers.tile([P, 2 * NQ, DC2, 2 * P], F8, tag="wqs")
        wks_t = pers.tile([P, 2 * NQ, DC2, 2 * P], F8, tag="wks")
        wv_t = pers.tile([P, 2, DC2, 2 * 512], F8, tag="wv")
        wo_t = pers.tile([P, NP2, 2 * D], F8, tag="wof")
        w2_t = pers.tile([P, FC2 // 2, 2, 2 * D], F8, tag="w2")
        if affine:
            g1b_t = pers.tile([P, D], F32, tag="g1b")
            be1b_t = pers.tile([P, D], F32, tag="be1b")
            g2b_t = pers.tile([P, D], F32, tag="g2b")
            be2b_t = pers.tile([P, D], F32, tag="be2b")
        else:
            g1b_t = be1b_t = g2b_t = be2b_t = None

        # startup DMAs, ordered for earliest first-exp: the DMA engine pool
        # is serially occupied, so issue exactly what unblocks Q/K/V first.
        nc.scalar.dma_start(bqs_t[:], bqs[:])
        nc.scalar.dma_start(bks_t[:], bks[:])
        nc.sync.dma_start(xTq_t[:], xTq8.rearrange("(c p) t -> p c t", p=P))
        nc.sync.dma_start(
            wqs_t[:], wqs.rearrange("p b (c m) -> p b c m", c=DC2))
        nc.sync.dma_start(
            wks_t[:], wks.rearrange("p b (c m) -> p b c m", c=DC2))
        for blk in range(4):
            nc.sync.dma_start(
                xT_t[:, :, blk * 512:(blk + 1) * 512],
                xT8.rearrange("(c p) t -> p c t", p=P)[
                    :, :, blk * 512:(blk + 1) * 512])
        nc.sync.dma_start(
            wv_t[:], wv8.rearrange("p (v c m) -> p v c m", v=2, c=DC2))
        nc.scalar.dma_start(b1_t[:], b1[:])
        nc.scalar.dma_start(bvb_t[:], bvb[:])
        nc.scalar.dma_start(ones_t[:], vones[:])
        nc.scalar.dma_start(
            hT_t[:, DC:DC + 2, :],
            hc8.rearrange("p (k t) -> p k t", k=2))

        nc.vector.tensor_copy(
            v_aug[:].rearrange("p t (h c) -> p t h c", c=HD + 1)[:, :, :, HD],
            ones_t[:, :, None].to_broadcast([P, KT, H]))
        # mid-kernel loads, all needed only by the post-attention chunks
        nc.sync.dma_start(xq_s[:], xq.rearrange("(t p) d -> p t d", p=P))
        nc.sync.dma_start(wo_t[:], wo8.rearrange("p (j m) -> p j m", j=NP2))
        nc.sync.dma_start(w2_t[:], w28.rearrange("p s (g m) -> p s g m", g=2))
        if affine:
            nc.scalar.dma_start(g1b_t[:], g1b[:])
            nc.scalar.dma_start(be1b_t[:], be1b[:])
            nc.scalar.dma_start(g2b_t[:], g2b[:])
            nc.scalar.dma_start(be2b_t[:], be2b[:])

        stp = es.enter_context(tc.tile_pool(name="stp", bufs=2, space="PSUM"))
        pvp = es.enter_context(tc.tile_pool(name="pvp", bufs=2, space="PSUM"))
        ppp = es.enter_context(tc.tile_pool(name="ppp", bufs=2))
        atd = es.enter_context(tc.tile_pool(name="atd", bufs=1))
        lnp = es.enter_context(tc.tile_pool(name="lnp", bufs=2))
        fp = es.enter_context(tc.tile_pool(name="fp", bufs=2))
        ft = es.enter_context(tc.tile_pool(name="ft", bufs=2))
        kvp_es = ExitStack()
        kvp = kvp_es.enter_context(tc.tile_pool(name="kvp", bufs=2,
                                                space="PSUM"))

        def emit_qproj(g):
            for i in range(2):
                b = 2 * g + i
                ps = kvp.tile([P, 512], F32, tag="kv", name=f"qps_{b}")
                for d2 in range(DC2):
                    nc.tensor.matmul(
                        ps[:],
                        wqs_t[:, b, d2, :].rearrange("p (i m) -> p i m", i=2),
                        xTq_t[:, 2 * d2:2 * d2 + 2, :],
                        start=(d2 == 0), stop=(d2 == DC2 - 1), perf_mode=DR)
                nc.vector.tensor_scalar(
                    qT_all[:, g, i, :], ps[:],
                    bqs_t[:, b:b + 1], None, ALU.add)

        def emit_kproj(g, i):
            """kT_all[:, g, i, :] for one hd-half of head-quad g."""
            b = 2 * g + i
            for blk in range(4):     # 512-key blocks
                ps = kvp.tile([P, 512], F32, tag="kv",
                              name=f"kps_{g}_{i}_{blk}")
                for d2 in range(DC2):
                    nc.tensor.matmul(
                        ps[:],
                        wks_t[:, b, d2, :].rearrange("p (i m) -> p i m", i=2),
                        xT_t[:, 2 * d2:2 * d2 + 2,
                             blk * 512:(blk + 1) * 512],
                        start=(d2 == 0), stop=(d2 == DC2 - 1), perf_mode=DR)
                nc.vector.tensor_scalar(
                    kT_all[:, g, i, blk * 512:(blk + 1) * 512], ps[:],
                    bks_t[:, b:b + 1], None, ALU.add)

        def emit_vsub(vh, sub):
            """v_aug columns for v-half vh, key tiles 4*sub..4*sub+3."""
            for tt in range(4 * sub, 4 * sub + 4):
                ps = kvp.tile([P, 512], F32, tag="kv",
                              name=f"vps_{vh}_{tt}")
                for d2 in range(DC2):
                    nc.tensor.matmul(
                        ps[:],
                        xT_t[:, 2 * d2:2 * d2 + 2, tt * P:(tt + 1) * P],
                        wv_t[:, vh, d2, :].rearrange("p (i n) -> p i n", i=2),
                        start=(d2 == 0), stop=(d2 == DC2 - 1), perf_mode=DR)
                dst = v_aug[:, tt, :].rearrange(
                    "p (h c) -> p h c", c=HD + 1)[:, vh * 8:(vh + 1) * 8, 0:HD]
                nc.vector.tensor_tensor(
                    dst, ps[:].rearrange("p (h c) -> p h c", c=HD),
                    bvb_t[:, vh * 512:(vh + 1) * 512].rearrange(
                        "p (h c) -> p h c", c=HD),
                    ALU.add)

        emit_qproj(0)
        emit_kproj(0, 0)
        emit_kproj(0, 1)
        emit_vsub(0, 0)

        fillers = []

        def drain():
            if fillers:
                fillers.pop(0)()

        def emit_attn(qh, pr):
            g, j0 = pr // 2, 2 * (pr % 2)
            pvs = [pvp.tile([P, 512], F32, tag="pv", name=f"pv_{qh}_{pr}_{h2}")
                   for h2 in range(2)]
            for grp in range(KT // 2):
                st = stp.tile([P, 2, 2, 256], F32, tag="st",
                              name=f"st_{qh}_{pr}_{grp}")
                for h2 in range(2):
                    j = j0 + h2
                    rows = slice(32 * j, 32 * j + 32)
                    for k in range(2):
                        kt = 2 * grp + k
                        nc.tensor.matmul(
                            st[:, h2, k, :],
                            kT_all[rows, g, :, kt * P:(kt + 1) * P],
                            qT_all[rows, g, :, qh * QH:(qh + 1) * QH],
                            start=True, stop=True, perf_mode=DR,
                            tile_position=(32 * j, 0))
                pp = ppp.tile([P, 2, 2, 256], F8, tag="pp",
                              name=f"pp_{qh}_{pr}_{grp}")
                nc.scalar.activation(pp[:], st[:], AF.Exp,
                                     scale=0.125 / (WS * WS))
                for h2 in range(2):
                    h = 2 * pr + h2
                    nc.tensor.matmul(
                        pvs[h2][:HD + 1, :QH],
                        v_aug[:, 2 * grp:2 * grp + 2,
                              h * (HD + 1):(h + 1) * (HD + 1)],
                        pp[:, h2, :, :],
                        start=(grp == 0), stop=(grp == KT // 2 - 1),
                        perf_mode=DR)
                if grp in (0, 2, 4):
                    drain()
            for h2 in range(2):
                rows = slice(h2 * HD, h2 * HD + HD)
                # bounce PV to SBUF so the PSUM bank frees after one short
                # DVE copy instead of the whole recip/bcast/normalize chain
                # (the bank gates the next pair's PV accumulator).
                pvc = atd.tile([P, QH], F32, tag="pvc",
                               name=f"pvc_{qh}_{pr}_{h2}")
                nc.vector.tensor_copy(pvc[:HD + 1, :], pvs[h2][:HD + 1, :QH])
                den = atd.tile([1, QH], F32, tag="den",
                               name=f"den_{qh}_{pr}_{h2}")
                nc.vector.reciprocal(den[:], pvc[HD:HD + 1, :])
                denb = atd.tile([HD, QH], F32, tag="denb",
                                name=f"denb_{qh}_{pr}_{h2}")
                nc.gpsimd.partition_broadcast(denb[:], den[:])
                nc.gpsimd.tensor_tensor(
                    outSB[rows, pr, qh * QH:(qh + 1) * QH],
                    pvc[:HD, :], denb[:], ALU.mult)

        # ---------- post-attention chunk emitters (token-tile tg) ----------
        postp_es = ExitStack()
        postp = [None]

        def emit_O(qh, tt):
            tg = 2 * qh + tt
            for oc in range(2):
                ps = postp[0].tile([P, 512], F32, tag="post",
                                   name=f"ops_{tg}_{oc}")
                for j2 in range(NP2):
                    nc.tensor.matmul(
                        ps[:],
                        outSB[:, 2 * j2:2 * j2 + 2, tg * P:(tg + 1) * P],
                        wo_t[:, j2, :].rearrange(
                            "p (i o) -> p i o", i=2)[:, :, oc * 512:(oc + 1) * 512],
                        start=(j2 == 0), stop=(j2 == NP2 - 1), perf_mode=DR)
                nc.vector.tensor_tensor(
                    h_t[:, tg, oc * 512:(oc + 1) * 512], ps[:],
                    xq_s[:, tg, oc * 512:(oc + 1) * 512], ALU.add)

        def emit_LNT(qh, tt):
            tg = 2 * qh + tt
            _layernorm(nc, lnp, h_t[:, tg, :], h_t[:, tg, :],
                       g1b_t, be1b_t, affine)
            for dcg in range(2):
                tp = postp[0].tile([P, 512], F32, tag="post",
                                   name=f"tp_{tg}_{dcg}")
                for k in range(4):
                    dc = 4 * dcg + k
                    nc.tensor.transpose(
                        tp[:, k * P:(k + 1) * P],
                        h_t[:, tg, dc * P:(dc + 1) * P], ident[:])
                nc.vector.tensor_copy(
                    hT_t[:, 4 * dcg:4 * dcg + 4, tg * P:(tg + 1) * P],
                    tp[:].rearrange("p (k m) -> p k m", k=4))
            # fold the fc2 bias into the residual (after transposes read h)
            nc.vector.tensor_tensor(h_t[:, tg, :], h_t[:, tg, :],
                                    b2b_t[:], ALU.add)

        def emit_F1(qh, fcg, use_act=False, pre=None):
            if pre is not None:
                w1_t = pre
            else:
                w1_t = fp.tile([P, 4, DC2 + 1, 2 * P], F8, tag="w1s")
                nc.sync.dma_start(w1_t[:], w18[:, fcg, :].rearrange(
                    "p (f c m) -> p f c m", f=4, c=DC2 + 1))
            for u in range(2):
                ps = postp[0].tile([P, 512], F32, tag="post",
                                   name=f"f1_{qh}_{fcg}_{u}")
                for f in (2 * u, 2 * u + 1):
                    fc = 4 * fcg + f
                    for d2 in range(DC2 + 1):
                        nc.tensor.matmul(
                            ps[:, (f - 2 * u) * QH:(f - 2 * u + 1) * QH],
                            w1_t[:, f, d2, :].rearrange(
                                "p (i m) -> p i m", i=2),
                            hT_t[:, 2 * d2:2 * d2 + 2, qh * QH:(qh + 1) * QH],
                            start=(d2 == 0), stop=(d2 == DC2), perf_mode=DR,
                            skip_group_check=True)
                dst = ff1[:, 4 * fcg + 2 * u:4 * fcg + 2 * u + 2,
                          qh * QH:(qh + 1) * QH]
                src_ap = ps[:].rearrange("p (f n) -> p f n", f=2)
                if use_act:
                    nc.scalar.activation(dst, src_ap, AF.Relu)
                else:
                    nc.vector.tensor_scalar(dst, src_ap, 0.0, None, ALU.max)

        def emit_F2(qh, tt, oc, part=2):
            """part: 0 = first half of the ff contraction, 1 = second half
            (+LN2/store), 2 = whole thing in one chunk."""
            tg = 2 * qh + tt
            if oc == 0 and part != 1:
                _f2_t2[tg] = ft.tile([P, D], F32, tag="t2", name=f"t2_{tg}")
            lo = FC2 // 2 if part == 1 else 0
            hi = FC2 // 2 if part == 0 else FC2
            ps = postp[0].tile([P, 512], F32, tag="post",
                               name=f"f2_{tg}_{oc}_{part}")
            for f2 in range(lo, hi):
                s, g2 = f2 // 2, f2 % 2
                nc.tensor.matmul(
                    ps[:],
                    ff1[:, 2 * f2:2 * f2 + 2, tg * P:(tg + 1) * P],
                    w2_t[:, s, g2, :].rearrange(
                        "p (i o) -> p i o", i=2)[:, :, oc * 512:(oc + 1) * 512],
                    start=(f2 == lo), stop=(f2 == hi - 1), perf_mode=DR)
            t2 = _f2_t2[tg]
            acc = h_t[:, tg, oc * 512:(oc + 1) * 512] if part != 1 \
                else t2[:, oc * 512:(oc + 1) * 512]
            nc.vector.scalar_tensor_tensor(
                t2[:, oc * 512:(oc + 1) * 512], ps[:], 1.0 / (WS * WS),
                acc, ALU.mult, ALU.add)
            if oc == 1 and part != 0:
                _layernorm(nc, lnp, t2[:], t2[:], g2b_t, be2b_t, affine)
                nc.sync.dma_start(
                    y.rearrange("(t p) d -> p t d", p=P)[:, tg, :], t2[:])

        _f2_t2 = {}

        # ---------------- schedule ----------------
        fillers.extend([
            lambda: emit_vsub(0, 1), lambda: emit_vsub(0, 2),
            lambda: emit_vsub(0, 3),
            lambda: emit_qproj(1),
            lambda: emit_kproj(1, 0), lambda: emit_kproj(1, 1),
            lambda: emit_vsub(1, 0), lambda: emit_vsub(1, 1),
            lambda: emit_qproj(2),
            lambda: emit_kproj(2, 0), lambda: emit_kproj(2, 1),
            lambda: emit_vsub(1, 2), lambda: emit_vsub(1, 3),
            lambda: emit_qproj(3),
            lambda: emit_kproj(3, 0), lambda: emit_kproj(3, 1),
        ])
        for pr in range(NPAIR):
            emit_attn(0, pr)
        while fillers:
            drain()
        kvp_es.close()
        nc.scalar.dma_start(b2b_t[:], b2b[:])
        postp[0] = postp_es.enter_context(
            tc.tile_pool(name="postp", bufs=2, space="PSUM"))
        fillers.extend(
            [lambda tt=tt: emit_O(0, tt) for tt in range(2)] +
            [lambda tt=tt: emit_LNT(0, tt) for tt in range(2)] +
            [lambda f=f: emit_F1(0, f) for f in range(FC // 4)] +
            [lambda tt=tt, oc=oc: emit_F2(0, tt, oc)
             for tt in range(2) for oc in range(2)])
        for pr in range(NPAIR):
            emit_attn(1, pr)
        while fillers:
            drain()
        w1_pre = {}
        for f in range(2):
            w1_pre[f] = fp.tile([P, 4, DC2 + 1, 2 * P], F8, tag="w1s",
                                name=f"w1pre_{f}")
            nc.sync.dma_start(w1_pre[f][:], w18[:, f, :].rearrange(
                "p (f c m) -> p f c m", f=4, c=DC2 + 1))
        for tt in range(2):
            emit_O(1, tt)
            emit_LNT(1, tt)
        for f in range(FC // 4):
            emit_F1(1, f, use_act=True, pre=w1_pre.get(f))
            if f == 4:
                for tt in range(2):
                    for oc in range(2):
                        emit_F2(1, tt, oc, part=0)
        for tt in range(2):
            for oc in range(2):
                emit_F2(1, tt, oc, part=1)
        postp_es.close()

    nc.compile()
    return nc


def _layernorm(nc, pool, dst, src, g_t, be_t, affine):
    """dst = (src - mean)/sqrt(var + eps) [* g + be], row-wise over 1024.

    bn_stats/bn_aggr produce mean+var in one DVE pass. rsqrt is computed
    as exp(-0.5*ln(v)) on ACT (both funcs live in one activation table,
    so no table thrash with the attention exps) and refined with one
    Newton step on DVE.
    """
    stats = pool.tile([P, 2, 6], F32, tag="ln_st")
    nc.vector.bn_stats(stats[:, 0, :], src[:, 0:D // 2])
    nc.vector.bn_stats(stats[:, 1, :], src[:, D // 2:D])
    mv = pool.tile([P, 2], F32, tag="ln_mv")
    nc.vector.bn_aggr(mv[:], stats[:])
    vv = pool.tile([P, 1], F32, tag="ln_v")
    nc.vector.tensor_scalar(vv[:], mv[:, 1:2], EPS, None, ALU.add)
    lnv = pool.tile([P, 1], F32, tag="ln_ln")
    nc.scalar.activation(lnv[:], vv[:], AF.Ln)
    r = pool.tile([P, 1], F32, tag="ln_r")
    nc.scalar.activation(r[:], lnv[:], AF.Exp, scale=-0.5)
    # one Newton step: r <- r * (1.5 - 0.5 * vv * r^2)
    t = pool.tile([P, 1], F32, tag="ln_t")
    nc.vector.tensor_tensor(t[:], r[:], r[:], ALU.mult)
    nc.vector.tensor_tensor(t[:], t[:], vv[:], ALU.mult)
    nc.vector.tensor_scalar(t[:], t[:], -0.5, 1.5, ALU.mult, ALU.add)
    nc.vector.tensor_tensor(r[:], r[:], t[:], ALU.mult)
    nc.vector.tensor_scalar(dst, src, mv[:, 0:1], r[:], ALU.subtract, ALU.mult)
    if affine:
        nc.vector.tensor_tensor(dst, dst, g_t[:], ALU.mult)
        nc.vector.tensor_tensor(dst, dst, be_t[:], ALU.add)


def _hc8():
    h = np.zeros((P, 2 * QT), np.float32)
    h[0, :QT] = WS
    return h.astype(mybir.dt.np(F8))


def make_in_maps(x, w_qkv, b_qkv, w_o, b_o, g1, be1, w1, b1, w2, b2, g2, be2):
    f = np.float32
    f8 = mybir.dt.np(F8)
    x = np.asarray(x, f)
    w_qkv = np.asarray(w_qkv, f)
    b_qkv = np.asarray(b_qkv, f)
    bc = lambda v: np.ascontiguousarray(
        np.broadcast_to(np.asarray(v, f).reshape(1, D), (P, D)))

    # [d, h*64+hd] -> [p][(g i)][d2 ik m] with m=32j+r -> (head 4g+j, hd 32i+r)
    def qk_split(w):
        t = (w * WS).reshape(DC2, 2, P, NQ, 4, 2, 32)
        t = t.transpose(2, 3, 5, 0, 1, 4, 6)    # [p, g, i, d2, ik, j, r]
        return np.ascontiguousarray(t.reshape(P, 2 * NQ, 2 * DC2 * P)).astype(f8)

    def bias_split(b):
        t = (b * WS).reshape(NQ, 4, 2, 32).transpose(1, 3, 0, 2)  # [j, r, g, i]
        return np.ascontiguousarray(t.reshape(P, 2 * NQ))

    wv_h = np.ascontiguousarray(
        (w_qkv[:, 2 * D:] * WS).reshape(DC2, 2, P, 2, 512)
        .transpose(2, 3, 0, 1, 4).reshape(P, 2 * DC2 * 2 * 512)).astype(f8)
    wo_h = np.ascontiguousarray(
        np.asarray(w_o, f).reshape(NP2, 2, P, D).transpose(2, 0, 1, 3)
        .reshape(P, NP2 * 2 * D)).astype(f8)
    w1_base = ((np.asarray(w1, f) * WS).reshape(DC2, 2, P, FC // 4, 4, P)
               .transpose(2, 3, 4, 0, 1, 5))          # [p, f4, f, d2, ik, m]
    w1_bias = np.zeros((P, FC // 4, 4, 1, 2, P), f)
    w1_bias[0, :, :, 0, 0, :] = np.asarray(b1, f).reshape(FC // 4, 4, P)
    w1_h = np.ascontiguousarray(
        np.concatenate([w1_base, w1_bias], axis=3)
        .reshape(P, FC // 4, 4 * (DC2 + 1) * 2 * P)).astype(f8)
    w2_h = np.ascontiguousarray(
        (np.asarray(w2, f) * WS).reshape(FC2 // 2, 2, 2, P, D)
        .transpose(3, 0, 1, 2, 4).reshape(P, FC2 // 2, 2 * 2 * D)).astype(f8)

    shared = {
        "wqs": qk_split(w_qkv[:, :D]),
        "wks": qk_split(w_qkv[:, D:2 * D]),
        "wv8": wv_h, "wo8": wo_h, "w18": w1_h, "w28": w2_h,
        "bqs": bias_split(b_qkv[:D]),
        "bks": bias_split(b_qkv[D:2 * D]),
        "b1": np.ascontiguousarray((np.asarray(b1, f) * WS).reshape(FC, P).T),
        "bvb": bc(np.asarray(b_qkv[2 * D:], f) * WS), "b2b": bc(b2),
        "g1b": bc(g1), "be1b": bc(be1), "g2b": bc(g2), "be2b": bc(be2),
        "vones": np.full((P, KT), WS, f).astype(f8),
        "hc8": _hc8(),
    }
    in_maps = []
    for c in range(8):
        n, qi = divmod(c, 4)
        xT8n = np.ascontiguousarray(x[n].T).astype(f8)
        m = dict(shared)
        m["xT8"] = xT8n
        m["xTq8"] = np.ascontiguousarray(xT8n[:, qi * QT:(qi + 1) * QT])
        m["xq"] = np.ascontiguousarray(x[n, qi * QT:(qi + 1) * QT, :]
                                       + np.asarray(b_o, f).reshape(1, D))
        in_maps.append(m)
    return in_maps


def get_nc(affine=True):
    if affine not in _CACHED_NC:
        _CACHED_NC[affine] = _build_nc(affine)
    return _CACHED_NC[affine]


def kernel(**inputs):
    in_maps = make_in_maps(**inputs)
    affine = not (np.all(np.asarray(inputs["g1"]) == 1)
                  and np.all(np.asarray(inputs["be1"]) == 0)
                  and np.all(np.asarray(inputs["g2"]) == 1)
                  and np.all(np.asarray(inputs["be2"]) == 0))
    nc = get_nc(affine)
    # The axon-proxied NRT occasionally reports a transient
    # NRT_EXEC_UNIT_UNRECOVERABLE on a cold first dispatch; a plain retry
    # has always succeeded with bit-identical results, so recover inline.
    last_err = None
    for _ in range(3):
        try:
            res = run_bass_kernel_spmd(nc, in_maps, list(range(8))).results
            break
        except Exception as e:  # noqa: BLE001
            last_err = e
    else:
        raise last_err
    yout = np.empty((NB, L, D), np.float32)
    for c in range(8):
        n, qi = divmod(c, 4)
        yout[n, qi * QT:(qi + 1) * QT] = res[c]["y"]
    return yout


if __name__ == "__main__":
    rng = np.random.default_rng(0)
    demo = {
        "x": rng.standard_normal((NB, L, D)).astype(np.float32),
        "w_qkv": rng.standard_normal((D, 3 * D)).astype(np.float32) * 0.03,
        "b_qkv": rng.standard_normal(3 * D).astype(np.float32) * 0.03,
        "w_o": rng.standard_normal((D, D)).astype(np.float32) * 0.03,
        "b_o": rng.standard_normal(D).astype(np.float32) * 0.03,
        "g1": np.ones(D, np.float32), "be1": np.zeros(D, np.float32),
        "w1": rng.standard_normal((D, FF)).astype(np.float32) * 0.03,
        "b1": rng.standard_normal(FF).astype(np.float32) * 0.03,
        "w2": rng.standard_normal((FF, D)).astype(np.float32) * 0.015,
        "b2": rng.standard_normal(D).astype(np.float32) * 0.015,
        "g2": np.ones(D, np.float32), "be2": np.zeros(D, np.float32),
    }
    out = kernel(**demo)
    print("kernel output:", out.shape, out.dtype, np.abs(out).mean())


# revision 23
# speedup vs baseline: 1.0485x; 1.0485x over previous
"""Trainium2 Bass kernel for a transformer encoder layer (nn_Encoder).

x:[2,2048,1024] f32. 8 NeuronCores, data-parallel: core c handles batch
n=c//4, query rows qi=c%4 (512 tokens). K/V are recomputed per core for
the full batch (x4 redundancy) to avoid collectives (~300us for the
8.4MB all-reduce this would replace).

All matmuls are fp8 e4m3 DoubleRow (2x PE column rate, 256-deep
contraction per instruction). K/Q are produced in a split-hd layout
([32 partitions, 2 k-tiles] per head) so even the hd=64 score matmuls
run DoubleRow. Weights are host-scaled x16 into the fp8 normal range;
the scale unwinds via the softmax ones-row (=16), the exp scale
(0.125/256) and a /256 on the fc2 PSUM. Residuals/LayerNorm stay f32.

The softmax exp stream on the Activation engine (~133us) is the
critical resource. Attention is split into two query-halves: while
half 1's exps run, half 0's output-proj/LN1/FFN execute in the PE/DVE
shadow (emitted as interleaved filler chunks); K/V projections for
later head-quads fill the shadow of half 0.
"""
import os
import sys
from contextlib import ExitStack

for _p in ("/opt/trn_rl_repo", "/root/.axon_site/_ro/trn_rl_repo"):
    if os.path.isdir(_p) and _p not in sys.path:
        sys.path.insert(0, _p)

import numpy as np
import concourse.bass as bass
import concourse.mybir as mybir
import concourse.tile as tile
from concourse import bacc
from concourse.bass_utils import run_bass_kernel_spmd
from concourse.masks import make_identity

F32 = mybir.dt.float32
F8 = mybir.dt.float8e4
AF = mybir.ActivationFunctionType
ALU = mybir.AluOpType
DR = mybir.MatmulPerfMode.DoubleRow

D = 1024
H = 16
HD = 64
FF = 4096
L = 2048
NB = 2
P = 128
QT = 512          # query tokens per core
QH = QT // 2      # query half
DC = D // P       # 8 chunks of the model dim
DC2 = DC // 2     # 4 DoubleRow chunk-pairs
KT = L // P       # 16 key tiles
FC = FF // P      # 32 ff chunks
FC2 = FC // 2     # 16 ff chunk-pairs
TT = QT // P      # 4 own-token tiles
NPAIR = H // 2    # 8 head pairs
NP2 = NPAIR // 2  # 4 pair-pairs
NQ = H // 4       # 4 head quads (scores split layout)
EPS = 1e-5
WS = 16.0         # host weight scale (fp8 range)

_CACHED_NC = {}


def _build_nc(affine=True):
    nc = bacc.Bacc("TRN2", target_bir_lowering=False)

    def dparam(name, shape, dt=F8):
        return nc.dram_tensor(name, shape, dt, kind="ExternalInput")

    xT8 = dparam("xT8", [D, L])            # x[n].T, fp8
    xTq8 = dparam("xTq8", [D, QT])         # own-token columns of xT, fp8
    xq = dparam("xq", [QT, D], F32)        # own tokens, natural (residual)
    # weights: partition-major fp8, DoubleRow k-tile-pair layouts
    wqs = dparam("wqs", [P, 2 * NQ, 2 * DC2 * P])    # [p][(g i)][d2 ik m]
    wks = dparam("wks", [P, 2 * NQ, 2 * DC2 * P])
    wv8 = dparam("wv8", [P, 2 * DC2 * 2 * 512])      # [p][vh d2 ik n]
    wo8 = dparam("wo8", [P, NP2 * 2 * D])            # [p][j ik o]
    w18 = dparam("w18", [P, FC // 4, 4 * (DC2 + 1) * 2 * P])  # [p][f4][f d2 ik m]
    w28 = dparam("w28", [P, FC2 // 2, 2 * 2 * D])        # [p][s][g ik o]
    bqs = dparam("bqs", [P, 2 * NQ], F32)  # x16 biases, split-hd order
    bks = dparam("bks", [P, 2 * NQ], F32)
    b1 = dparam("b1", [P, FC], F32)
    bvb = dparam("bvb", [P, D], F32)
    b2b = dparam("b2b", [P, D], F32)       # natural scale
    g1b = dparam("g1b", [P, D], F32)
    be1b = dparam("be1b", [P, D], F32)
    g2b = dparam("g2b", [P, D], F32)
    be2b = dparam("be2b", [P, D], F32)
    vones = dparam("vones", [P, KT])       # value 16 (denominator row)
    hc8 = dparam("hc8", [P, 2 * QT])       # F1 bias rows: [16,0...;0...]

    y = nc.dram_tensor("y", [QT, D], F32, kind="ExternalOutput")

    with tile.TileContext(nc) as tc, ExitStack() as es:
        # Pre-load the one activation table that serves every ACT func we
        # use (Exp for softmax, Ln+Exp for the LN rsqrt): without this the
        # first-fit chooser thrashes exp<->ln tables at 1283ns per load.
        from concourse.hw_specs import get_activation_tables
        _tabs = get_activation_tables(nc.m.arch)
        _idx = next(i for i, (_, s) in enumerate(_tabs.items())
                    if AF.Exp in s and AF.Ln in s)
        nc.scalar.add_instruction(mybir.InstLoadActFuncSet(
            name=nc.scalar.bass.get_next_instruction_name(),
            act_func_set_id=_idx, ins=[], outs=[]))

        pers = es.enter_context(tc.tile_pool(name="pers", bufs=1))
        ident = pers.tile([P, P], F32, tag="ident")
        make_identity(nc, ident[:])
        bqs_t = pers.tile([P, 2 * NQ], F32, tag="bqs")
        bks_t = pers.tile([P, 2 * NQ], F32, tag="bks")
        b1_t = pers.tile([P, FC], F32, tag="b1")
        bvb_t = pers.tile([P, D], F32, tag="bvb")
        # b2b reuses bvb's slot: bvb is dead after V-proj, long before
        # the LNT chunks fold b2 into the residual.
        b2b_t = pers.tile([P, D], F32, tag="bvb", name="b2b_t")

        xT_t = pers.tile([P, DC, L], F8, tag="xT")
        xTq_t = pers.tile([P, DC, QT], F8, tag="xTq")
        v_aug = pers.tile([P, KT, H * (HD + 1)], F8, tag="vaug")
        ones_t = pers.tile([P, KT], F8, tag="ones")
        qT_all = pers.tile([P, NQ, 2, QT], F8, tag="qT")
        kT_all = pers.tile([P, NQ, 2, L], F8, tag="kT")
        outSB = pers.tile([P, NPAIR, QT], F8, tag="outSB")
        h_t = pers.tile([P, TT, D], F32, tag="h")
        hT_t = pers.tile([P, DC + 2, QT], F8, tag="hT")
        ff1 = pers.tile([P, FC, QT], F8, tag="ff1")
        xq_s = pers.tile([P, TT, D], F32, tag="xqs")
        wqs_t = pers.tile([P, 2 * NQ, DC2, 2 * P], F8, tag="wqs")
        wks_t = pers.tile([P, 2 * NQ, DC2, 2 * P], F8, tag="wks")
        wv_t = pers.tile([P, 2, DC2, 2 * 512], F8, tag="wv")
        wo_t = pers.tile([P, NP2, 2 * D], F8, tag="wof")
        w2_t = pers.tile([P, FC2 // 2, 2, 2 * D], F8, tag="w2")
        if affine:
            g1b_t = pers.tile([P, D], F32, tag="g1b")
            be1b_t = pers.tile([P, D], F32, tag="be1b")
            g2b_t = pers.tile([P, D], F32, tag="g2b")
            be2b_t = pers.tile([P, D], F32, tag="be2b")
        else:
            g1b_t = be1b_t = g2b_t = be2b_t = None

        # startup DMAs, ordered for earliest first-exp: the DMA engine pool
        # is serially occupied, so issue exactly what unblocks Q/K/V first.
        nc.scalar.dma_start(bqs_t[:], bqs[:])
        nc.scalar.dma_start(bks_t[:], bks[:])
        nc.sync.dma_start(xTq_t[:], xTq8.rearrange("(c p) t -> p c t", p=P))
        nc.sync.dma_start(
            wqs_t[:], wqs.rearrange("p b (c m) -> p b c m", c=DC2))
        nc.sync.dma_start(
            wks_t[:], wks.rearrange("p b (c m) -> p b c m", c=DC2))
        for blk in range(4):
            nc.sync.dma_start(
                xT_t[:, :, blk * 512:(blk + 1) * 512],
                xT8.rearrange("(c p) t -> p c t", p=P)[
                    :, :, blk * 512:(blk + 1) * 512])
        nc.sync.dma_start(
            wv_t[:], wv8.rearrange("p (v c m) -> p v c m", v=2, c=DC2))
        nc.scalar.dma_start(b1_t[:], b1[:])
        nc.scalar.dma_start(bvb_t[:], bvb[:])
        nc.scalar.dma_start(ones_t[:], vones[:])
        nc.scalar.dma_start(
            hT_t[:, DC:DC + 2, :],
            hc8.rearrange("p (k t) -> p k t", k=2))

        nc.vector.tensor_copy(
            v_aug[:].rearrange("p t (h c) -> p t h c", c=HD + 1)[:, :, :, HD],
            ones_t[:, :, None].to_broadcast([P, KT, H]))
        # mid-kernel loads, all needed only by the post-attention chunks
        nc.sync.dma_start(xq_s[:], xq.rearrange("(t p) d -> p t d", p=P))
        nc.sync.dma_start(wo_t[:], wo8.rearrange("p (j m) -> p j m", j=NP2))
        nc.sync.dma_start(w2_t[:], w28.rearrange("p s (g m) -> p s g m", g=2))
        if affine:
            nc.scalar.dma_start(g1b_t[:], g1b[:])
            nc.scalar.dma_start(be1b_t[:], be1b[:])
            nc.scalar.dma_start(g2b_t[:], g2b[:])
            nc.scalar.dma_start(be2b_t[:], be2b[:])

        stp = es.enter_context(tc.tile_pool(name="stp", bufs=2, space="PSUM"))
        pvp = es.enter_context(tc.tile_pool(name="pvp", bufs=2, space="PSUM"))
        ppp = es.enter_context(tc.tile_pool(name="ppp", bufs=3))
        atd = es.enter_context(tc.tile_pool(name="atd", bufs=1))
        lnp = es.enter_context(tc.tile_pool(name="lnp", bufs=2))
        fp = es.enter_context(tc.tile_pool(name="fp", bufs=2))
        ft = es.enter_context(tc.tile_pool(name="ft", bufs=2))
        kvp_es = ExitStack()
        kvp = kvp_es.enter_context(tc.tile_pool(name="kvp", bufs=2,
                                                space="PSUM"))

        def emit_qproj(g):
            for i in range(2):
                b = 2 * g + i
                ps = kvp.tile([P, 512], F32, tag="kv", name=f"qps_{b}")
                for d2 in range(DC2):
                    nc.tensor.matmul(
                        ps[:],
                        wqs_t[:, b, d2, :].rearrange("p (i m) -> p i m", i=2),
                        xTq_t[:, 2 * d2:2 * d2 + 2, :],
                        start=(d2 == 0), stop=(d2 == DC2 - 1), perf_mode=DR)
                nc.vector.tensor_scalar(
                    qT_all[:, g, i, :], ps[:],
                    bqs_t[:, b:b + 1], None, ALU.add)

        def emit_kproj(g, i):
            """kT_all[:, g, i, :] for one hd-half of head-quad g."""
            b = 2 * g + i
            for blk in range(4):     # 512-key blocks
                ps = kvp.tile([P, 512], F32, tag="kv",
                              name=f"kps_{g}_{i}_{blk}")
                for d2 in range(DC2):
                    nc.tensor.matmul(
                        ps[:],
                        wks_t[:, b, d2, :].rearrange("p (i m) -> p i m", i=2),
                        xT_t[:, 2 * d2:2 * d2 + 2,
                             blk * 512:(blk + 1) * 512],
                        start=(d2 == 0), stop=(d2 == DC2 - 1), perf_mode=DR)
                nc.vector.tensor_scalar(
                    kT_all[:, g, i, blk * 512:(blk + 1) * 512], ps[:],
                    bks_t[:, b:b + 1], None, ALU.add)

        def emit_vsub(vh, sub):
            """v_aug columns for v-half vh, key tiles 4*sub..4*sub+3."""
            for tt in range(4 * sub, 4 * sub + 4):
                ps = kvp.tile([P, 512], F32, tag="kv",
                              name=f"vps_{vh}_{tt}")
                for d2 in range(DC2):
                    nc.tensor.matmul(
                        ps[:],
                        xT_t[:, 2 * d2:2 * d2 + 2, tt * P:(tt + 1) * P],
                        wv_t[:, vh, d2, :].rearrange("p (i n) -> p i n", i=2),
                        start=(d2 == 0), stop=(d2 == DC2 - 1), perf_mode=DR)
                dst = v_aug[:, tt, :].rearrange(
                    "p (h c) -> p h c", c=HD + 1)[:, vh * 8:(vh + 1) * 8, 0:HD]
                nc.vector.tensor_tensor(
                    dst, ps[:].rearrange("p (h c) -> p h c", c=HD),
                    bvb_t[:, vh * 512:(vh + 1) * 512].rearrange(
                        "p (h c) -> p h c", c=HD),
                    ALU.add)

        emit_qproj(0)
        emit_kproj(0, 0)
        emit_kproj(0, 1)
        emit_vsub(0, 0)

        fillers = []

        def drain():
            if fillers:
                fillers.pop(0)()

        def emit_attn(qh, pr):
            g, j0 = pr // 2, 2 * (pr % 2)
            pvs = [pvp.tile([P, 512], F32, tag="pv", name=f"pv_{qh}_{pr}_{h2}")
                   for h2 in range(2)]
            for grp in range(KT // 2):
                st = stp.tile([P, 2, 2, 256], F32, tag="st",
                              name=f"st_{qh}_{pr}_{grp}")
                for h2 in range(2):
                    j = j0 + h2
                    rows = slice(32 * j, 32 * j + 32)
                    for k in range(2):
                        kt = 2 * grp + k
                        nc.tensor.matmul(
                            st[:, h2, k, :],
                            kT_all[rows, g, :, kt * P:(kt + 1) * P],
                            qT_all[rows, g, :, qh * QH:(qh + 1) * QH],
                            start=True, stop=True, perf_mode=DR,
                            tile_position=(32 * j, 0))
                pp = ppp.tile([P, 2, 2, 256], F8, tag="pp",
                              name=f"pp_{qh}_{pr}_{grp}")
                nc.scalar.activation(pp[:], st[:], AF.Exp,
                                     scale=0.125 / (WS * WS))
                for h2 in range(2):
                    h = 2 * pr + h2
                    nc.tensor.matmul(
                        pvs[h2][:HD + 1, :QH],
                        v_aug[:, 2 * grp:2 * grp + 2,
                              h * (HD + 1):(h + 1) * (HD + 1)],
                        pp[:, h2, :, :],
                        start=(grp == 0), stop=(grp == KT // 2 - 1),
                        perf_mode=DR)
                if grp in (0, 2, 4):
                    drain()
            for h2 in range(2):
                rows = slice(h2 * HD, h2 * HD + HD)
                # bounce PV to SBUF so the PSUM bank frees after one short
                # DVE copy instead of the whole recip/bcast/normalize chain
                # (the bank gates the next pair's PV accumulator).
                pvc = atd.tile([P, QH], F32, tag="pvc",
                               name=f"pvc_{qh}_{pr}_{h2}")
                nc.vector.tensor_copy(pvc[:HD + 1, :], pvs[h2][:HD + 1, :QH])
                den = atd.tile([1, QH], F32, tag="den",
                               name=f"den_{qh}_{pr}_{h2}")
                nc.vector.reciprocal(den[:], pvc[HD:HD + 1, :])
                denb = atd.tile([HD, QH], F32, tag="denb",
                                name=f"denb_{qh}_{pr}_{h2}")
                nc.gpsimd.partition_broadcast(denb[:], den[:])
                nc.gpsimd.tensor_tensor(
                    outSB[rows, pr, qh * QH:(qh + 1) * QH],
                    pvc[:HD, :], denb[:], ALU.mult)

        # ---------- post-attention chunk emitters (token-tile tg) ----------
        postp_es = ExitStack()
        postp = [None]

        def emit_O(qh, tt):
            tg = 2 * qh + tt
            for oc in range(2):
                ps = postp[0].tile([P, 512], F32, tag="post",
                                   name=f"ops_{tg}_{oc}")
                for j2 in range(NP2):
                    nc.tensor.matmul(
                        ps[:],
                        outSB[:, 2 * j2:2 * j2 + 2, tg * P:(tg + 1) * P],
                        wo_t[:, j2, :].rearrange(
                            "p (i o) -> p i o", i=2)[:, :, oc * 512:(oc + 1) * 512],
                        start=(j2 == 0), stop=(j2 == NP2 - 1), perf_mode=DR)
                nc.vector.tensor_tensor(
                    h_t[:, tg, oc * 512:(oc + 1) * 512], ps[:],
                    xq_s[:, tg, oc * 512:(oc + 1) * 512], ALU.add)

        def emit_LNT(qh, tt):
            tg = 2 * qh + tt
            _layernorm(nc, lnp, h_t[:, tg, :], h_t[:, tg, :],
                       g1b_t, be1b_t, affine)
            for dcg in range(2):
                tp = postp[0].tile([P, 512], F32, tag="post",
                                   name=f"tp_{tg}_{dcg}")
                for k in range(4):
                    dc = 4 * dcg + k
                    nc.tensor.transpose(
                        tp[:, k * P:(k + 1) * P],
                        h_t[:, tg, dc * P:(dc + 1) * P], ident[:])
                nc.vector.tensor_copy(
                    hT_t[:, 4 * dcg:4 * dcg + 4, tg * P:(tg + 1) * P],
                    tp[:].rearrange("p (k m) -> p k m", k=4))
            # fold the fc2 bias into the residual (after transposes read h)
            nc.vector.tensor_tensor(h_t[:, tg, :], h_t[:, tg, :],
                                    b2b_t[:], ALU.add)

        def emit_F1(qh, fcg, use_act=False, pre=None):
            if pre is not None:
                w1_t = pre
            else:
                w1_t = fp.tile([P, 4, DC2 + 1, 2 * P], F8, tag="w1s")
                nc.sync.dma_start(w1_t[:], w18[:, fcg, :].rearrange(
                    "p (f c m) -> p f c m", f=4, c=DC2 + 1))
            for u in range(2):
                ps = postp[0].tile([P, 512], F32, tag="post",
                                   name=f"f1_{qh}_{fcg}_{u}")
                for f in (2 * u, 2 * u + 1):
                    fc = 4 * fcg + f
                    for d2 in range(DC2 + 1):
                        nc.tensor.matmul(
                            ps[:, (f - 2 * u) * QH:(f - 2 * u + 1) * QH],
                            w1_t[:, f, d2, :].rearrange(
                                "p (i m) -> p i m", i=2),
                            hT_t[:, 2 * d2:2 * d2 + 2, qh * QH:(qh + 1) * QH],
                            start=(d2 == 0), stop=(d2 == DC2), perf_mode=DR,
                            skip_group_check=True)
                dst = ff1[:, 4 * fcg + 2 * u:4 * fcg + 2 * u + 2,
                          qh * QH:(qh + 1) * QH]
                src_ap = ps[:].rearrange("p (f n) -> p f n", f=2)
                if use_act:
                    nc.scalar.activation(dst, src_ap, AF.Relu)
                else:
                    nc.vector.tensor_scalar(dst, src_ap, 0.0, None, ALU.max)

        def emit_F2(qh, tt, oc, part=2):
            """part: 0 = first half of the ff contraction, 1 = second half
            (+LN2/store), 2 = whole thing in one chunk."""
            tg = 2 * qh + tt
            if oc == 0 and part != 1:
                _f2_t2[tg] = ft.tile([P, D], F32, tag="t2", name=f"t2_{tg}")
            lo = FC2 // 2 if part == 1 else 0
            hi = FC2 // 2 if part == 0 else FC2
            ps = postp[0].tile([P, 512], F32, tag="post",
                               name=f"f2_{tg}_{oc}_{part}")
            for f2 in range(lo, hi):
                s, g2 = f2 // 2, f2 % 2
                nc.tensor.matmul(
                    ps[:],
                    ff1[:, 2 * f2:2 * f2 + 2, tg * P:(tg + 1) * P],
                    w2_t[:, s, g2, :].rearrange(
                        "p (i o) -> p i o", i=2)[:, :, oc * 512:(oc + 1) * 512],
                    start=(f2 == lo), stop=(f2 == hi - 1), perf_mode=DR)
            t2 = _f2_t2[tg]
            acc = h_t[:, tg, oc * 512:(oc + 1) * 512] if part != 1 \
                else t2[:, oc * 512:(oc + 1) * 512]
            nc.vector.scalar_tensor_tensor(
                t2[:, oc * 512:(oc + 1) * 512], ps[:], 1.0 / (WS * WS),
                acc, ALU.mult, ALU.add)
            if oc == 1 and part != 0:
                _layernorm(nc, lnp, t2[:], t2[:], g2b_t, be2b_t, affine)
                nc.sync.dma_start(
                    y.rearrange("(t p) d -> p t d", p=P)[:, tg, :], t2[:])

        _f2_t2 = {}

        # ---------------- schedule ----------------
        fillers.extend([
            lambda: emit_vsub(0, 1), lambda: emit_vsub(0, 2),
            lambda: emit_vsub(0, 3),
            lambda: emit_qproj(1),
            lambda: emit_kproj(1, 0), lambda: emit_kproj(1, 1),
            lambda: emit_vsub(1, 0), lambda: emit_vsub(1, 1),
            lambda: emit_qproj(2),
            lambda: emit_kproj(2, 0), lambda: emit_kproj(2, 1),
            lambda: emit_vsub(1, 2), lambda: emit_vsub(1, 3),
            lambda: emit_qproj(3),
            lambda: emit_kproj(3, 0), lambda: emit_kproj(3, 1),
        ])
        for pr in range(NPAIR):
            emit_attn(0, pr)
        while fillers:
            drain()
        kvp_es.close()
        nc.scalar.dma_start(b2b_t[:], b2b[:])
        postp[0] = postp_es.enter_context(
            tc.tile_pool(name="postp", bufs=2, space="PSUM"))
        fillers.extend(
            [lambda tt=tt: emit_O(0, tt) for tt in range(2)] +
            [lambda tt=tt: emit_LNT(0, tt) for tt in range(2)] +
            [lambda f=f: emit_F1(0, f) for f in range(FC // 4)] +
            [lambda tt=tt, oc=oc: emit_F2(0, tt, oc)
             for tt in range(2) for oc in range(2)])
        for pr in range(NPAIR):
            emit_attn(1, pr)
        while fillers:
            drain()
        w1_pre = {}
        for f in range(2):
            w1_pre[f] = fp.tile([P, 4, DC2 + 1, 2 * P], F8, tag="w1s",
                                name=f"w1pre_{f}")
            nc.sync.dma_start(w1_pre[f][:], w18[:, f, :].rearrange(
                "p (f c m) -> p f c m", f=4, c=DC2 + 1))
        for tt in range(2):
            emit_O(1, tt)
            emit_LNT(1, tt)
        for f in range(FC // 4):
            emit_F1(1, f, use_act=True, pre=w1_pre.get(f))
            if f == 4:
                for tt in range(2):
                    for oc in range(2):
                        emit_F2(1, tt, oc, part=0)
        for tt in range(2):
            for oc in range(2):
                emit_F2(1, tt, oc, part=1)
        postp_es.close()

    nc.compile()
    return nc


def _layernorm(nc, pool, dst, src, g_t, be_t, affine):
    """dst = (src - mean)/sqrt(var + eps) [* g + be], row-wise over 1024.

    bn_stats/bn_aggr produce mean+var in one DVE pass. rsqrt is computed
    as exp(-0.5*ln(v)) on ACT (both funcs live in one activation table,
    so no table thrash with the attention exps) and refined with one
    Newton step on DVE.
    """
    stats = pool.tile([P, 2, 6], F32, tag="ln_st")
    nc.vector.bn_stats(stats[:, 0, :], src[:, 0:D // 2])
    nc.vector.bn_stats(stats[:, 1, :], src[:, D // 2:D])
    mv = pool.tile([P, 2], F32, tag="ln_mv")
    nc.vector.bn_aggr(mv[:], stats[:])
    vv = pool.tile([P, 1], F32, tag="ln_v")
    nc.vector.tensor_scalar(vv[:], mv[:, 1:2], EPS, None, ALU.add)
    lnv = pool.tile([P, 1], F32, tag="ln_ln")
    nc.scalar.activation(lnv[:], vv[:], AF.Ln)
    r = pool.tile([P, 1], F32, tag="ln_r")
    nc.scalar.activation(r[:], lnv[:], AF.Exp, scale=-0.5)
    # one Newton step: r <- r * (1.5 - 0.5 * vv * r^2)
    t = pool.tile([P, 1], F32, tag="ln_t")
    nc.vector.tensor_tensor(t[:], r[:], r[:], ALU.mult)
    nc.vector.tensor_tensor(t[:], t[:], vv[:], ALU.mult)
    nc.vector.tensor_scalar(t[:], t[:], -0.5, 1.5, ALU.mult, ALU.add)
    nc.vector.tensor_tensor(r[:], r[:], t[:], ALU.mult)
    nc.vector.tensor_scalar(dst, src, mv[:, 0:1], r[:], ALU.subtract, ALU.mult)
    if affine:
        nc.vector.tensor_tensor(dst, dst, g_t[:], ALU.mult)
        nc.vector.tensor_tensor(dst, dst, be_t[:], ALU.add)


def _hc8():
    h = np.zeros((P, 2 * QT), np.float32)
    h[0, :QT] = WS
    return h.astype(mybir.dt.np(F8))


def make_in_maps(x, w_qkv, b_qkv, w_o, b_o, g1, be1, w1, b1, w2, b2, g2, be2):
    f = np.float32
    f8 = mybir.dt.np(F8)
    x = np.asarray(x, f)
    w_qkv = np.asarray(w_qkv, f)
    b_qkv = np.asarray(b_qkv, f)
    bc = lambda v: np.ascontiguousarray(
        np.broadcast_to(np.asarray(v, f).reshape(1, D), (P, D)))

    # [d, h*64+hd] -> [p][(g i)][d2 ik m] with m=32j+r -> (head 4g+j, hd 32i+r)
    def qk_split(w):
        t = (w * WS).reshape(DC2, 2, P, NQ, 4, 2, 32)
        t = t.transpose(2, 3, 5, 0, 1, 4, 6)    # [p, g, i, d2, ik, j, r]
        return np.ascontiguousarray(t.reshape(P, 2 * NQ, 2 * DC2 * P)).astype(f8)

    def bias_split(b):
        t = (b * WS).reshape(NQ, 4, 2, 32).transpose(1, 3, 0, 2)  # [j, r, g, i]
        return np.ascontiguousarray(t.reshape(P, 2 * NQ))

    wv_h = np.ascontiguousarray(
        (w_qkv[:, 2 * D:] * WS).reshape(DC2, 2, P, 2, 512)
        .transpose(2, 3, 0, 1, 4).reshape(P, 2 * DC2 * 2 * 512)).astype(f8)
    wo_h = np.ascontiguousarray(
        np.asarray(w_o, f).reshape(NP2, 2, P, D).transpose(2, 0, 1, 3)
        .reshape(P, NP2 * 2 * D)).astype(f8)
    w1_base = ((np.asarray(w1, f) * WS).reshape(DC2, 2, P, FC // 4, 4, P)
               .transpose(2, 3, 4, 0, 1, 5))          # [p, f4, f, d2, ik, m]
    w1_bias = np.zeros((P, FC // 4, 4, 1, 2, P), f)
    w1_bias[0, :, :, 0, 0, :] = np.asarray(b1, f).reshape(FC // 4, 4, P)
    w1_h = np.ascontiguousarray(
        np.concatenate([w1_base, w1_bias], axis=3)
        .reshape(P, FC // 4, 4 * (DC2 + 1) * 2 * P)).astype(f8)
    w2_h = np.ascontiguousarray(
        (np.asarray(w2, f) * WS).reshape(FC2 // 2, 2, 2, P, D)
        .transpose(3, 0, 1, 2, 4).reshape(P, FC2 // 2, 2 * 2 * D)).astype(f8)

    shared = {
        "wqs": qk_split(w_qkv[:, :D]),
        "wks": qk_split(w_qkv[:, D:2 * D]),
        "wv8": wv_h, "wo8": wo_h, "w18": w1_h, "w28": w2_h,
        "bqs": bias_split(b_qkv[:D]),
        "bks": bias_split(b_qkv[D:2 * D]),
        "b1": np.ascontiguousarray((np.asarray(b1, f) * WS).reshape(FC, P).T),
        "bvb": bc(np.asarray(b_qkv[2 * D:], f) * WS), "b2b": bc(b2),
        "g1b": bc(g1), "be1b": bc(be1), "g2b": bc(g2), "be2b": bc(be2),
        "vones": np.full((P, KT), WS, f).astype(f8),
        "hc8": _hc8(),
    }
    in_maps = []
    for c in range(8):
        n, qi = divmod(c, 4)
        xT8n = np.ascontiguousarray(x[n].T).astype(f8)
        m = dict(shared)
        m["xT8"] = xT8n
        m["xTq8"] = np.ascontiguousarray(xT8n[:, qi * QT:(qi + 1) * QT])
        m["xq"] = np.ascontiguousarray(x[n, qi * QT:(qi + 1) * QT, :]
                                       + np.asarray(b_o, f).reshape(1, D))
        in_maps.append(m)
    return in_maps


def get_nc(affine=True):
    if affine not in _CACHED_NC:
        _CACHED_NC[affine] = _build_nc(affine)
    return _CACHED_NC[affine]


def kernel(**inputs):
    in_maps = make_in_maps(**inputs)
    affine = not (np.all(np.asarray(inputs["g1"]) == 1)
                  and np.all(np.asarray(inputs["be1"]) == 0)
                  and np.all(np.asarray(inputs["g2"]) == 1)
                  and np.all(np.asarray(inputs["be2"]) == 0))
    nc = get_nc(affine)
    # The axon-proxied NRT occasionally reports a transient
    # NRT_EXEC_UNIT_UNRECOVERABLE on a cold first dispatch; a plain retry
    # has always succeeded with bit-identical results, so recover inline.
    last_err = None
    for _ in range(3):
        try:
            res = run_bass_kernel_spmd(nc, in_maps, list(range(8))).results
            break
        except Exception as e:  # noqa: BLE001
            last_err = e
    else:
        raise last_err
    yout = np.empty((NB, L, D), np.float32)
    for c in range(8):
        n, qi = divmod(c, 4)
        yout[n, qi * QT:(qi + 1) * QT] = res[c]["y"]
    return yout


if __name__ == "__main__":
    rng = np.random.default_rng(0)
    demo = {
        "x": rng.standard_normal((NB, L, D)).astype(np.float32),
        "w_qkv": rng.standard_normal((D, 3 * D)).astype(np.float32) * 0.03,
        "b_qkv": rng.standard_normal(3 * D).astype(np.float32) * 0.03,
        "w_o": rng.standard_normal((D, D)).astype(np.float32) * 0.03,
        "b_o": rng.standard_normal(D).astype(np.float32) * 0.03,
        "g1": np.ones(D, np.float32), "be1": np.zeros(D, np.float32),
        "w1": rng.standard_normal((D, FF)).astype(np.float32) * 0.03,
        "b1": rng.standard_normal(FF).astype(np.float32) * 0.03,
        "w2": rng.standard_normal((FF, D)).astype(np.float32) * 0.015,
        "b2": rng.standard_normal(D).astype(np.float32) * 0.015,
        "g2": np.ones(D, np.float32), "be2": np.zeros(D, np.float32),
    }
    out = kernel(**demo)
    print("kernel output:", out.shape, out.dtype, np.abs(out).mean())


# revision 25
# speedup vs baseline: 1.1023x; 1.0513x over previous
"""Trainium2 Bass kernel for a transformer encoder layer (nn_Encoder).

x:[2,2048,1024] f32. 8 NeuronCores, data-parallel: core c handles batch
n=c//4, query rows qi=c%4 (512 tokens). K/V are recomputed per core for
the full batch (x4 redundancy) to avoid collectives (~300us for the
8.4MB all-reduce this would replace).

All matmuls are fp8 e4m3 DoubleRow (2x PE column rate, 256-deep
contraction per instruction). K/Q are produced in a split-hd layout
([32 partitions, 2 k-tiles] per head) so even the hd=64 score matmuls
run DoubleRow. Weights are host-scaled x16 into the fp8 normal range;
the scale unwinds via the softmax ones-row (=16), the exp scale
(0.125/256) and a /256 on the fc2 PSUM. Residuals/LayerNorm stay f32.

The softmax exp stream on the Activation engine (~133us) is the
critical resource. Attention is split into two query-halves: while
half 1's exps run, half 0's output-proj/LN1/FFN execute in the PE/DVE
shadow (emitted as interleaved filler chunks); K/V projections for
later head-quads fill the shadow of half 0.
"""
import os
import sys
from contextlib import ExitStack

for _p in ("/opt/trn_rl_repo", "/root/.axon_site/_ro/trn_rl_repo"):
    if os.path.isdir(_p) and _p not in sys.path:
        sys.path.insert(0, _p)

import numpy as np
import concourse.bass as bass
import concourse.mybir as mybir
import concourse.tile as tile
from concourse import bacc
from concourse.bass_utils import run_bass_kernel_spmd
from concourse.masks import make_identity

F32 = mybir.dt.float32
F8 = mybir.dt.float8e4
AF = mybir.ActivationFunctionType
ALU = mybir.AluOpType
DR = mybir.MatmulPerfMode.DoubleRow

D = 1024
H = 16
HD = 64
FF = 4096
L = 2048
NB = 2
P = 128
QT = 512          # query tokens per core
QH = QT // 2      # query half
DC = D // P       # 8 chunks of the model dim
DC2 = DC // 2     # 4 DoubleRow chunk-pairs
KT = L // P       # 16 key tiles
FC = FF // P      # 32 ff chunks
FC2 = FC // 2     # 16 ff chunk-pairs
TT = QT // P      # 4 own-token tiles
NPAIR = H // 2    # 8 head pairs
NP2 = NPAIR // 2  # 4 pair-pairs
NQ = H // 4       # 4 head quads (scores split layout)
EPS = 1e-5
WS = 16.0         # host weight scale (fp8 range)

_CACHED_NC = {}


def _build_nc(affine=True):
    nc = bacc.Bacc("TRN2", target_bir_lowering=False)

    def dparam(name, shape, dt=F8):
        return nc.dram_tensor(name, shape, dt, kind="ExternalInput")

    xT8 = dparam("xT8", [D, L])            # x[n].T, fp8
    xTq8 = dparam("xTq8", [D, QT])         # own-token columns of xT, fp8
    xq = dparam("xq", [QT, D], F32)        # own tokens, natural (residual)
    # weights: partition-major fp8, DoubleRow k-tile-pair layouts
    wqs = dparam("wqs", [P, 2 * NQ, 2 * DC2 * P])    # [p][(g i)][d2 ik m]
    wks = dparam("wks", [P, 2 * NQ, 2 * DC2 * P])
    wv8 = dparam("wv8", [P, 2 * DC2 * 2 * 512])      # [p][vh d2 ik n]
    wo8 = dparam("wo8", [P, NP2 * 2 * D])            # [p][j ik o]
    w18 = dparam("w18", [P, FC // 4, 4 * (DC2 + 1) * 2 * P])  # [p][f4][f d2 ik m]
    w28 = dparam("w28", [P, FC2 // 2, 2 * 2 * D])        # [p][s][g ik o]
    bqs = dparam("bqs", [P, 2 * NQ], F32)  # x16 biases, split-hd order
    bks = dparam("bks", [P, 2 * NQ], F32)
    b1 = dparam("b1", [P, FC], F32)
    bvb = dparam("bvb", [P, D], F32)
    b2b = dparam("b2b", [P, D], F32)       # natural scale
    g1b = dparam("g1b", [P, D], F32)
    be1b = dparam("be1b", [P, D], F32)
    g2b = dparam("g2b", [P, D], F32)
    be2b = dparam("be2b", [P, D], F32)
    vones = dparam("vones", [P, KT])       # value 16 (denominator row)
    hc8 = dparam("hc8", [P, 2 * QT])       # F1 bias rows: [16,0...;0...]

    y = nc.dram_tensor("y", [QT, D], F32, kind="ExternalOutput")

    with tile.TileContext(nc) as tc, ExitStack() as es:
        # Pre-load the one activation table that serves every ACT func we
        # use (Exp for softmax, Ln+Exp for the LN rsqrt): without this the
        # first-fit chooser thrashes exp<->ln tables at 1283ns per load.
        from concourse.hw_specs import get_activation_tables
        _tabs = get_activation_tables(nc.m.arch)
        _idx = next(i for i, (_, s) in enumerate(_tabs.items())
                    if AF.Exp in s and AF.Ln in s)
        nc.scalar.add_instruction(mybir.InstLoadActFuncSet(
            name=nc.scalar.bass.get_next_instruction_name(),
            act_func_set_id=_idx, ins=[], outs=[]))

        pers = es.enter_context(tc.tile_pool(name="pers", bufs=1))
        ident = pers.tile([P, P], F32, tag="ident")
        make_identity(nc, ident[:])
        bqs_t = pers.tile([P, 2 * NQ], F32, tag="bqs")
        bks_t = pers.tile([P, 2 * NQ], F32, tag="bks")
        b1_t = pers.tile([P, FC], F32, tag="b1")
        bvb_t = pers.tile([P, D], F32, tag="bvb")
        # b2b reuses bvb's slot: bvb is dead after V-proj, long before
        # the LNT chunks fold b2 into the residual.
        b2b_t = pers.tile([P, D], F32, tag="bvb", name="b2b_t")

        xT_t = pers.tile([P, DC, L], F8, tag="xT")
        xTq_t = pers.tile([P, DC, QT], F8, tag="xTq")
        v_aug = pers.tile([P, KT, H * (HD + 1)], F8, tag="vaug")
        ones_t = pers.tile([P, KT], F8, tag="ones")
        qT_all = pers.tile([P, NQ, 2, QT], F8, tag="qT")
        kT_all = pers.tile([P, NQ, 2, L], F8, tag="kT")
        outSB = pers.tile([P, NPAIR, QT], F8, tag="outSB")
        h_t = pers.tile([P, TT, D], F32, tag="h")
        hT_t = pers.tile([P, DC + 2, QT], F8, tag="hT")
        ff1 = pers.tile([P, FC, QT], F8, tag="ff1")
        xq_s = pers.tile([P, TT, D], F32, tag="xqs")
        wqs_t = pers.tile([P, 2 * NQ, DC2, 2 * P], F8, tag="wqs")
        wks_t = pers.tile([P, 2 * NQ, DC2, 2 * P], F8, tag="wks")
        wv_t = pers.tile([P, 2, DC2, 2 * 512], F8, tag="wv")
        wo_t = pers.tile([P, NP2, 2 * D], F8, tag="wof")
        w2_t = pers.tile([P, FC2 // 2, 2, 2 * D], F8, tag="w2")
        if affine:
            g1b_t = pers.tile([P, D], F32, tag="g1b")
            be1b_t = pers.tile([P, D], F32, tag="be1b")
            g2b_t = pers.tile([P, D], F32, tag="g2b")
            be2b_t = pers.tile([P, D], F32, tag="be2b")
        else:
            g1b_t = be1b_t = g2b_t = be2b_t = None

        # startup DMAs, ordered for earliest first-exp: the DMA engine pool
        # is serially occupied, so issue exactly what unblocks Q/K/V first.
        nc.scalar.dma_start(bqs_t[:], bqs[:])
        nc.scalar.dma_start(bks_t[:], bks[:])
        nc.sync.dma_start(xTq_t[:], xTq8.rearrange("(c p) t -> p c t", p=P))
        wqs_r = wqs.rearrange("p b (c m) -> p b c m", c=DC2)
        wks_r = wks.rearrange("p b (c m) -> p b c m", c=DC2)
        nc.sync.dma_start(wqs_t[:, 0:2], wqs_r[:, 0:2])
        nc.sync.dma_start(wks_t[:, 0:2], wks_r[:, 0:2])
        for blk in range(4):
            nc.sync.dma_start(
                xT_t[:, :, blk * 512:(blk + 1) * 512],
                xT8.rearrange("(c p) t -> p c t", p=P)[
                    :, :, blk * 512:(blk + 1) * 512])
        wv_r = wv8.rearrange("p (v c m) -> p v c m", v=2, c=DC2)
        nc.sync.dma_start(wv_t[:, 0:1], wv_r[:, 0:1])
        nc.sync.dma_start(wqs_t[:, 2:8], wqs_r[:, 2:8])
        nc.sync.dma_start(wks_t[:, 2:8], wks_r[:, 2:8])
        nc.sync.dma_start(wv_t[:, 1:2], wv_r[:, 1:2])
        nc.scalar.dma_start(b1_t[:], b1[:])
        nc.scalar.dma_start(bvb_t[:], bvb[:])
        nc.scalar.dma_start(ones_t[:], vones[:])
        nc.scalar.dma_start(
            hT_t[:, DC:DC + 2, :],
            hc8.rearrange("p (k t) -> p k t", k=2))

        nc.vector.tensor_copy(
            v_aug[:].rearrange("p t (h c) -> p t h c", c=HD + 1)[:, :, :, HD],
            ones_t[:, :, None].to_broadcast([P, KT, H]))
        # mid-kernel loads, all needed only by the post-attention chunks
        nc.sync.dma_start(xq_s[:], xq.rearrange("(t p) d -> p t d", p=P))
        nc.sync.dma_start(wo_t[:], wo8.rearrange("p (j m) -> p j m", j=NP2))
        nc.sync.dma_start(w2_t[:], w28.rearrange("p s (g m) -> p s g m", g=2))
        if affine:
            nc.scalar.dma_start(g1b_t[:], g1b[:])
            nc.scalar.dma_start(be1b_t[:], be1b[:])
            nc.scalar.dma_start(g2b_t[:], g2b[:])
            nc.scalar.dma_start(be2b_t[:], be2b[:])

        stp = es.enter_context(tc.tile_pool(name="stp", bufs=2, space="PSUM"))
        pvp = es.enter_context(tc.tile_pool(name="pvp", bufs=2, space="PSUM"))
        ppp = es.enter_context(tc.tile_pool(name="ppp", bufs=3))
        atd = es.enter_context(tc.tile_pool(name="atd", bufs=2))
        lnp = es.enter_context(tc.tile_pool(name="lnp", bufs=2))
        fp = es.enter_context(tc.tile_pool(name="fp", bufs=2))
        ft = es.enter_context(tc.tile_pool(name="ft", bufs=2))
        kvp_es = ExitStack()
        kvp = kvp_es.enter_context(tc.tile_pool(name="kvp", bufs=2,
                                                space="PSUM"))

        def emit_qproj(g):
            for i in range(2):
                b = 2 * g + i
                ps = kvp.tile([P, 512], F32, tag="kv", name=f"qps_{b}")
                for d2 in range(DC2):
                    nc.tensor.matmul(
                        ps[:],
                        wqs_t[:, b, d2, :].rearrange("p (i m) -> p i m", i=2),
                        xTq_t[:, 2 * d2:2 * d2 + 2, :],
                        start=(d2 == 0), stop=(d2 == DC2 - 1), perf_mode=DR)
                nc.vector.tensor_scalar(
                    qT_all[:, g, i, :], ps[:],
                    bqs_t[:, b:b + 1], None, ALU.add)

        def emit_kproj(g, i):
            """kT_all[:, g, i, :] for one hd-half of head-quad g."""
            b = 2 * g + i
            for blk in range(4):     # 512-key blocks
                ps = kvp.tile([P, 512], F32, tag="kv",
                              name=f"kps_{g}_{i}_{blk}")
                for d2 in range(DC2):
                    nc.tensor.matmul(
                        ps[:],
                        wks_t[:, b, d2, :].rearrange("p (i m) -> p i m", i=2),
                        xT_t[:, 2 * d2:2 * d2 + 2,
                             blk * 512:(blk + 1) * 512],
                        start=(d2 == 0), stop=(d2 == DC2 - 1), perf_mode=DR)
                nc.vector.tensor_scalar(
                    kT_all[:, g, i, blk * 512:(blk + 1) * 512], ps[:],
                    bks_t[:, b:b + 1], None, ALU.add)

        def emit_vsub(vh, sub):
            """v_aug columns for v-half vh, key tiles 4*sub..4*sub+3."""
            for tt in range(4 * sub, 4 * sub + 4):
                ps = kvp.tile([P, 512], F32, tag="kv",
                              name=f"vps_{vh}_{tt}")
                for d2 in range(DC2):
                    nc.tensor.matmul(
                        ps[:],
                        xT_t[:, 2 * d2:2 * d2 + 2, tt * P:(tt + 1) * P],
                        wv_t[:, vh, d2, :].rearrange("p (i n) -> p i n", i=2),
                        start=(d2 == 0), stop=(d2 == DC2 - 1), perf_mode=DR)
                dst = v_aug[:, tt, :].rearrange(
                    "p (h c) -> p h c", c=HD + 1)[:, vh * 8:(vh + 1) * 8, 0:HD]
                nc.vector.tensor_tensor(
                    dst, ps[:].rearrange("p (h c) -> p h c", c=HD),
                    bvb_t[:, vh * 512:(vh + 1) * 512].rearrange(
                        "p (h c) -> p h c", c=HD),
                    ALU.add)

        emit_qproj(0)
        emit_kproj(0, 0)
        emit_kproj(0, 1)
        emit_vsub(0, 0)

        fillers = []

        def drain():
            if fillers:
                fillers.pop(0)()

        def emit_attn(qh, pr):
            g, j0 = pr // 2, 2 * (pr % 2)
            pvs = [pvp.tile([P, 512], F32, tag="pv", name=f"pv_{qh}_{pr}_{h2}")
                   for h2 in range(2)]
            for grp in range(KT // 2):
                st = stp.tile([P, 2, 2, 256], F32, tag="st",
                              name=f"st_{qh}_{pr}_{grp}")
                for h2 in range(2):
                    j = j0 + h2
                    rows = slice(32 * j, 32 * j + 32)
                    for k in range(2):
                        kt = 2 * grp + k
                        nc.tensor.matmul(
                            st[:, h2, k, :],
                            kT_all[rows, g, :, kt * P:(kt + 1) * P],
                            qT_all[rows, g, :, qh * QH:(qh + 1) * QH],
                            start=True, stop=True, perf_mode=DR,
                            tile_position=(32 * j, 0))
                pp = ppp.tile([P, 2, 2, 256], F8, tag="pp",
                              name=f"pp_{qh}_{pr}_{grp}")
                nc.scalar.activation(pp[:], st[:], AF.Exp,
                                     scale=0.125 / (WS * WS))
                for h2 in range(2):
                    h = 2 * pr + h2
                    nc.tensor.matmul(
                        pvs[h2][:HD + 1, :QH],
                        v_aug[:, 2 * grp:2 * grp + 2,
                              h * (HD + 1):(h + 1) * (HD + 1)],
                        pp[:, h2, :, :],
                        start=(grp == 0), stop=(grp == KT // 2 - 1),
                        perf_mode=DR)
                if grp in (0, 2, 4):
                    drain()
            for h2 in range(2):
                rows = slice(h2 * HD, h2 * HD + HD)
                # bounce PV to SBUF so the PSUM bank frees after one short
                # DVE copy instead of the whole recip/bcast/normalize chain
                # (the bank gates the next pair's PV accumulator).
                pvc = atd.tile([P, QH], F32, tag="pvc",
                               name=f"pvc_{qh}_{pr}_{h2}")
                nc.vector.tensor_copy(pvc[:HD + 1, :], pvs[h2][:HD + 1, :QH])
                # reciprocal in place over the denominator row of pvc
                nc.vector.reciprocal(pvc[HD:HD + 1, :], pvc[HD:HD + 1, :])
                denb = atd.tile([HD, QH], F32, tag="denb",
                                name=f"denb_{qh}_{pr}_{h2}")
                nc.gpsimd.partition_broadcast(denb[:], pvc[HD:HD + 1, :])
                nc.gpsimd.tensor_tensor(
                    outSB[rows, pr, qh * QH:(qh + 1) * QH],
                    pvc[:HD, :], denb[:], ALU.mult)

        # ---------- post-attention chunk emitters (token-tile tg) ----------
        postp_es = ExitStack()
        postp = [None]

        def emit_O(qh, tt):
            tg = 2 * qh + tt
            for oc in range(2):
                ps = postp[0].tile([P, 512], F32, tag="post",
                                   name=f"ops_{tg}_{oc}")
                for j2 in range(NP2):
                    nc.tensor.matmul(
                        ps[:],
                        outSB[:, 2 * j2:2 * j2 + 2, tg * P:(tg + 1) * P],
                        wo_t[:, j2, :].rearrange(
                            "p (i o) -> p i o", i=2)[:, :, oc * 512:(oc + 1) * 512],
                        start=(j2 == 0), stop=(j2 == NP2 - 1), perf_mode=DR)
                nc.vector.tensor_tensor(
                    h_t[:, tg, oc * 512:(oc + 1) * 512], ps[:],
                    xq_s[:, tg, oc * 512:(oc + 1) * 512], ALU.add)

        def emit_LNT(qh, tt):
            tg = 2 * qh + tt
            _layernorm(nc, lnp, h_t[:, tg, :], h_t[:, tg, :],
                       g1b_t, be1b_t, affine)
            for dcg in range(2):
                tp = postp[0].tile([P, 512], F32, tag="post",
                                   name=f"tp_{tg}_{dcg}")
                for k in range(4):
                    dc = 4 * dcg + k
                    nc.tensor.transpose(
                        tp[:, k * P:(k + 1) * P],
                        h_t[:, tg, dc * P:(dc + 1) * P], ident[:])
                nc.vector.tensor_copy(
                    hT_t[:, 4 * dcg:4 * dcg + 4, tg * P:(tg + 1) * P],
                    tp[:].rearrange("p (k m) -> p k m", k=4))
            # fold the fc2 bias into the residual (after transposes read h)
            nc.gpsimd.tensor_tensor(h_t[:, tg, :], h_t[:, tg, :],
                                    b2b_t[:], ALU.add)

        def emit_F1(qh, fcg, use_act=False, pre=None, split_tt=False):
            if pre is not None:
                w1_t = pre
            else:
                w1_t = fp.tile([P, 4, DC2 + 1, 2 * P], F8, tag="w1s")
                nc.sync.dma_start(w1_t[:], w18[:, fcg, :].rearrange(
                    "p (f c m) -> p f c m", f=4, c=DC2 + 1))
            tparts = ((0, 1) if split_tt else (None,))
            for u in range(2):
                ps = postp[0].tile([P, 512], F32, tag="post",
                                   name=f"f1_{qh}_{fcg}_{u}")
                for tl in tparts:
                    cols = slice(qh * QH, (qh + 1) * QH) if tl is None else \
                        slice((2 * qh + tl) * P, (2 * qh + tl + 1) * P)
                    w = QH if tl is None else P
                    for f in (2 * u, 2 * u + 1):
                        fl = f - 2 * u
                        base = fl * QH + (0 if tl is None else tl * P)
                        for d2 in range(DC2 + 1):
                            nc.tensor.matmul(
                                ps[:, base:base + w],
                                w1_t[:, f, d2, :].rearrange(
                                    "p (i m) -> p i m", i=2),
                                hT_t[:, 2 * d2:2 * d2 + 2, cols],
                                start=(d2 == 0), stop=(d2 == DC2),
                                perf_mode=DR, skip_group_check=True)
                    dst = ff1[:, 4 * fcg + 2 * u:4 * fcg + 2 * u + 2, cols]
                    src_ap = ps[:].rearrange("p (f n) -> p f n", f=2) \
                        if tl is None else \
                        ps[:].rearrange("p (f n) -> p f n", f=2)[
                            :, :, tl * P:(tl + 1) * P]
                    if use_act:
                        nc.scalar.activation(dst, src_ap, AF.Relu)
                    else:
                        nc.vector.tensor_scalar(dst, src_ap, 0.0, None,
                                                ALU.max)

        def emit_F2(qh, tt, oc, part=2):
            """part: 0 = first half of the ff contraction, 1 = second half
            (+LN2/store), 2 = whole thing in one chunk."""
            tg = 2 * qh + tt
            if oc == 0 and part != 1:
                _f2_t2[tg] = ft.tile([P, D], F32, tag="t2", name=f"t2_{tg}")
            lo = FC2 // 2 if part == 1 else 0
            hi = FC2 // 2 if part == 0 else FC2
            ps = postp[0].tile([P, 512], F32, tag="post",
                               name=f"f2_{tg}_{oc}_{part}")
            for f2 in range(lo, hi):
                s, g2 = f2 // 2, f2 % 2
                nc.tensor.matmul(
                    ps[:],
                    ff1[:, 2 * f2:2 * f2 + 2, tg * P:(tg + 1) * P],
                    w2_t[:, s, g2, :].rearrange(
                        "p (i o) -> p i o", i=2)[:, :, oc * 512:(oc + 1) * 512],
                    start=(f2 == lo), stop=(f2 == hi - 1), perf_mode=DR)
            t2 = _f2_t2[tg]
            acc = h_t[:, tg, oc * 512:(oc + 1) * 512] if part != 1 \
                else t2[:, oc * 512:(oc + 1) * 512]
            nc.vector.scalar_tensor_tensor(
                t2[:, oc * 512:(oc + 1) * 512], ps[:], 1.0 / (WS * WS),
                acc, ALU.mult, ALU.add)
            if oc == 1 and part != 0:
                _layernorm(nc, lnp, t2[:], t2[:], g2b_t, be2b_t, affine)
                nc.sync.dma_start(
                    y.rearrange("(t p) d -> p t d", p=P)[:, tg, :], t2[:])

        _f2_t2 = {}

        # ---------------- schedule ----------------
        fillers.extend([
            lambda: emit_vsub(0, 1), lambda: emit_vsub(0, 2),
            lambda: emit_vsub(0, 3),
            lambda: emit_qproj(1),
            lambda: emit_kproj(1, 0), lambda: emit_kproj(1, 1),
            lambda: emit_vsub(1, 0), lambda: emit_vsub(1, 1),
            lambda: emit_qproj(2),
            lambda: emit_kproj(2, 0), lambda: emit_kproj(2, 1),
            lambda: emit_vsub(1, 2), lambda: emit_vsub(1, 3),
            lambda: emit_qproj(3),
            lambda: emit_kproj(3, 0), lambda: emit_kproj(3, 1),
        ])
        for pr in range(NPAIR):
            emit_attn(0, pr)
        while fillers:
            drain()
        kvp_es.close()
        nc.scalar.dma_start(b2b_t[:], b2b[:])
        postp[0] = postp_es.enter_context(
            tc.tile_pool(name="postp", bufs=2, space="PSUM"))
        fillers.extend(
            [lambda tt=tt: emit_O(0, tt) for tt in range(2)] +
            [lambda tt=tt: emit_LNT(0, tt) for tt in range(2)] +
            [lambda f=f: emit_F1(0, f) for f in range(FC // 4)] +
            [lambda tt=tt, oc=oc: emit_F2(0, tt, oc)
             for tt in range(2) for oc in range(2)])
        for pr in range(NPAIR):
            emit_attn(1, pr)
        while fillers:
            drain()
        w1_pre = {}
        for f in range(2):
            w1_pre[f] = fp.tile([P, 4, DC2 + 1, 2 * P], F8, tag="w1s",
                                name=f"w1pre_{f}")
            nc.sync.dma_start(w1_pre[f][:], w18[:, f, :].rearrange(
                "p (f c m) -> p f c m", f=4, c=DC2 + 1))
        for tt in range(2):
            emit_O(1, tt)
            emit_LNT(1, tt)
        for f in range(FC // 4):
            emit_F1(1, f, use_act=True, pre=w1_pre.get(f), split_tt=True)
            if f == 4:
                for tt in range(2):
                    for oc in range(2):
                        emit_F2(1, tt, oc, part=0)
        for tt in range(2):
            for oc in range(2):
                emit_F2(1, tt, oc, part=1)
        postp_es.close()

    nc.compile()
    return nc


def _layernorm(nc, pool, dst, src, g_t, be_t, affine):
    """dst = (src - mean)/sqrt(var + eps) [* g + be], row-wise over 1024.

    bn_stats/bn_aggr produce mean+var in one DVE pass. rsqrt is computed
    as exp(-0.5*ln(v)) on ACT (both funcs live in one activation table,
    so no table thrash with the attention exps) and refined with one
    Newton step on DVE.
    """
    stats = pool.tile([P, 2, 6], F32, tag="ln_st")
    nc.vector.bn_stats(stats[:, 0, :], src[:, 0:D // 2])
    nc.vector.bn_stats(stats[:, 1, :], src[:, D // 2:D])
    mv = pool.tile([P, 2], F32, tag="ln_mv")
    nc.vector.bn_aggr(mv[:], stats[:])
    vv = pool.tile([P, 1], F32, tag="ln_v")
    nc.vector.tensor_scalar(vv[:], mv[:, 1:2], EPS, None, ALU.add)
    lnv = pool.tile([P, 1], F32, tag="ln_ln")
    nc.scalar.activation(lnv[:], vv[:], AF.Ln)
    r = pool.tile([P, 1], F32, tag="ln_r")
    nc.scalar.activation(r[:], lnv[:], AF.Exp, scale=-0.5)
    # one Newton step: r <- r * (1.5 - 0.5 * vv * r^2)
    t = pool.tile([P, 1], F32, tag="ln_t")
    nc.vector.tensor_tensor(t[:], r[:], r[:], ALU.mult)
    nc.vector.tensor_tensor(t[:], t[:], vv[:], ALU.mult)
    nc.vector.tensor_scalar(t[:], t[:], -0.5, 1.5, ALU.mult, ALU.add)
    nc.vector.tensor_tensor(r[:], r[:], t[:], ALU.mult)
    nc.vector.tensor_scalar(dst, src, mv[:, 0:1], r[:], ALU.subtract, ALU.mult)
    if affine:
        nc.vector.tensor_tensor(dst, dst, g_t[:], ALU.mult)
        nc.vector.tensor_tensor(dst, dst, be_t[:], ALU.add)


def _hc8():
    h = np.zeros((P, 2 * QT), np.float32)
    h[0, :QT] = WS
    return h.astype(mybir.dt.np(F8))


def make_in_maps(x, w_qkv, b_qkv, w_o, b_o, g1, be1, w1, b1, w2, b2, g2, be2):
    f = np.float32
    f8 = mybir.dt.np(F8)
    x = np.asarray(x, f)
    w_qkv = np.asarray(w_qkv, f)
    b_qkv = np.asarray(b_qkv, f)
    bc = lambda v: np.ascontiguousarray(
        np.broadcast_to(np.asarray(v, f).reshape(1, D), (P, D)))

    # [d, h*64+hd] -> [p][(g i)][d2 ik m] with m=32j+r -> (head 4g+j, hd 32i+r)
    def qk_split(w):
        t = (w * WS).reshape(DC2, 2, P, NQ, 4, 2, 32)
        t = t.transpose(2, 3, 5, 0, 1, 4, 6)    # [p, g, i, d2, ik, j, r]
        return np.ascontiguousarray(t.reshape(P, 2 * NQ, 2 * DC2 * P)).astype(f8)

    def bias_split(b):
        t = (b * WS).reshape(NQ, 4, 2, 32).transpose(1, 3, 0, 2)  # [j, r, g, i]
        return np.ascontiguousarray(t.reshape(P, 2 * NQ))

    wv_h = np.ascontiguousarray(
        (w_qkv[:, 2 * D:] * WS).reshape(DC2, 2, P, 2, 512)
        .transpose(2, 3, 0, 1, 4).reshape(P, 2 * DC2 * 2 * 512)).astype(f8)
    wo_h = np.ascontiguousarray(
        np.asarray(w_o, f).reshape(NP2, 2, P, D).transpose(2, 0, 1, 3)
        .reshape(P, NP2 * 2 * D)).astype(f8)
    w1_base = ((np.asarray(w1, f) * WS).reshape(DC2, 2, P, FC // 4, 4, P)
               .transpose(2, 3, 4, 0, 1, 5))          # [p, f4, f, d2, ik, m]
    w1_bias = np.zeros((P, FC // 4, 4, 1, 2, P), f)
    w1_bias[0, :, :, 0, 0, :] = np.asarray(b1, f).reshape(FC // 4, 4, P)
    w1_h = np.ascontiguousarray(
        np.concatenate([w1_base, w1_bias], axis=3)
        .reshape(P, FC // 4, 4 * (DC2 + 1) * 2 * P)).astype(f8)
    w2_h = np.ascontiguousarray(
        (np.asarray(w2, f) * WS).reshape(FC2 // 2, 2, 2, P, D)
        .transpose(3, 0, 1, 2, 4).reshape(P, FC2 // 2, 2 * 2 * D)).astype(f8)

    shared = {
        "wqs": qk_split(w_qkv[:, :D]),
        "wks": qk_split(w_qkv[:, D:2 * D]),
        "wv8": wv_h, "wo8": wo_h, "w18": w1_h, "w28": w2_h,
        "bqs": bias_split(b_qkv[:D]),
        "bks": bias_split(b_qkv[D:2 * D]),
        "b1": np.ascontiguousarray((np.asarray(b1, f) * WS).reshape(FC, P).T),
        "bvb": bc(np.asarray(b_qkv[2 * D:], f) * WS), "b2b": bc(b2),
        "g1b": bc(g1), "be1b": bc(be1), "g2b": bc(g2), "be2b": bc(be2),
        "vones": np.full((P, KT), WS, f).astype(f8),
        "hc8": _hc8(),
    }
    in_maps = []
    for c in range(8):
        n, qi = divmod(c, 4)
        xT8n = np.ascontiguousarray(x[n].T).astype(f8)
        m = dict(shared)
        m["xT8"] = xT8n
        m["xTq8"] = np.ascontiguousarray(xT8n[:, qi * QT:(qi + 1) * QT])
        m["xq"] = np.ascontiguousarray(x[n, qi * QT:(qi + 1) * QT, :]
                                       + np.asarray(b_o, f).reshape(1, D))
        in_maps.append(m)
    return in_maps


def get_nc(affine=True):
    if affine not in _CACHED_NC:
        _CACHED_NC[affine] = _build_nc(affine)
    return _CACHED_NC[affine]


def kernel(**inputs):
    in_maps = make_in_maps(**inputs)
    affine = not (np.all(np.asarray(inputs["g1"]) == 1)
                  and np.all(np.asarray(inputs["be1"]) == 0)
                  and np.all(np.asarray(inputs["g2"]) == 1)
                  and np.all(np.asarray(inputs["be2"]) == 0))
    nc = get_nc(affine)
    # The axon-proxied NRT occasionally reports a transient
    # NRT_EXEC_UNIT_UNRECOVERABLE on a cold first dispatch; a plain retry
    # has always succeeded with bit-identical results, so recover inline.
    last_err = None
    for _ in range(3):
        try:
            res = run_bass_kernel_spmd(nc, in_maps, list(range(8))).results
            break
        except Exception as e:  # noqa: BLE001
            last_err = e
    else:
        raise last_err
    yout = np.empty((NB, L, D), np.float32)
    for c in range(8):
        n, qi = divmod(c, 4)
        yout[n, qi * QT:(qi + 1) * QT] = res[c]["y"]
    return yout


if __name__ == "__main__":
    rng = np.random.default_rng(0)
    demo = {
        "x": rng.standard_normal((NB, L, D)).astype(np.float32),
        "w_qkv": rng.standard_normal((D, 3 * D)).astype(np.float32) * 0.03,
        "b_qkv": rng.standard_normal(3 * D).astype(np.float32) * 0.03,
        "w_o": rng.standard_normal((D, D)).astype(np.float32) * 0.03,
        "b_o": rng.standard_normal(D).astype(np.float32) * 0.03,
        "g1": np.ones(D, np.float32), "be1": np.zeros(D, np.float32),
        "w1": rng.standard_normal((D, FF)).astype(np.float32) * 0.03,
        "b1": rng.standard_normal(FF).astype(np.float32) * 0.03,
        "w2": rng.standard_normal((FF, D)).astype(np.float32) * 0.015,
        "b2": rng.standard_normal(D).astype(np.float32) * 0.015,
        "g2": np.ones(D, np.float32), "be2": np.zeros(D, np.float32),
    }
    out = kernel(**demo)
    print("kernel output:", out.shape, out.dtype, np.abs(out).mean())


# revision 26
# speedup vs baseline: 1.1332x; 1.0281x over previous
"""Trainium2 Bass kernel for a transformer encoder layer (nn_Encoder).

x:[2,2048,1024] f32. 8 NeuronCores, data-parallel: core c handles batch
n=c//4, query rows qi=c%4 (512 tokens). K/V are recomputed per core for
the full batch (x4 redundancy) to avoid collectives (~300us for the
8.4MB all-reduce this would replace).

All matmuls are fp8 e4m3 DoubleRow (2x PE column rate, 256-deep
contraction per instruction). K/Q are produced in a split-hd layout
([32 partitions, 2 k-tiles] per head) so even the hd=64 score matmuls
run DoubleRow. Weights are host-scaled x16 into the fp8 normal range;
the scale unwinds via the softmax ones-row (=16), the exp scale
(0.125/256) and a /256 on the fc2 PSUM. Residuals/LayerNorm stay f32.

The softmax exp stream on the Activation engine (~133us) is the
critical resource. Attention is split into two query-halves: while
half 1's exps run, half 0's output-proj/LN1/FFN execute in the PE/DVE
shadow (emitted as interleaved filler chunks); K/V projections for
later head-quads fill the shadow of half 0.
"""
import os
import sys
from contextlib import ExitStack

for _p in ("/opt/trn_rl_repo", "/root/.axon_site/_ro/trn_rl_repo"):
    if os.path.isdir(_p) and _p not in sys.path:
        sys.path.insert(0, _p)

import numpy as np
import concourse.bass as bass
import concourse.mybir as mybir
import concourse.tile as tile
from concourse import bacc
from concourse.bass_utils import run_bass_kernel_spmd
from concourse.masks import make_identity

F32 = mybir.dt.float32
F8 = mybir.dt.float8e4
AF = mybir.ActivationFunctionType
ALU = mybir.AluOpType
DR = mybir.MatmulPerfMode.DoubleRow

D = 1024
H = 16
HD = 64
FF = 4096
L = 2048
NB = 2
P = 128
QT = 512          # query tokens per core
QH = QT // 2      # query half
DC = D // P       # 8 chunks of the model dim
DC2 = DC // 2     # 4 DoubleRow chunk-pairs
KT = L // P       # 16 key tiles
FC = FF // P      # 32 ff chunks
FC2 = FC // 2     # 16 ff chunk-pairs
TT = QT // P      # 4 own-token tiles
NPAIR = H // 2    # 8 head pairs
NP2 = NPAIR // 2  # 4 pair-pairs
NQ = H // 4       # 4 head quads (scores split layout)
EPS = 1e-5
WS = 16.0         # host weight scale (fp8 range)

_CACHED_NC = {}


def _build_nc(affine=True):
    nc = bacc.Bacc("TRN2", target_bir_lowering=False)

    def dparam(name, shape, dt=F8):
        return nc.dram_tensor(name, shape, dt, kind="ExternalInput")

    xT8 = dparam("xT8", [D, L])            # x[n].T, fp8
    xTq8 = dparam("xTq8", [D, QT])         # own-token columns of xT, fp8
    xq = dparam("xq", [QT, D], F32)        # own tokens, natural (residual)
    # weights: partition-major fp8, DoubleRow k-tile-pair layouts
    wqs = dparam("wqs", [P, 2 * NQ, 2 * DC2 * P])    # [p][(g i)][d2 ik m]
    wks = dparam("wks", [P, 2 * NQ, 2 * DC2 * P])
    wv8 = dparam("wv8", [P, 2 * DC2 * 2 * 512])      # [p][vh d2 ik n]
    wo8 = dparam("wo8", [P, NP2 * 2 * D])            # [p][j ik o]
    w18 = dparam("w18", [P, FC // 4, 4 * (DC2 + 1) * 2 * P])  # [p][f4][f d2 ik m]
    w28 = dparam("w28", [P, FC2 // 2, 2 * 2 * D])        # [p][s][g ik o]
    bqs = dparam("bqs", [P, 2 * NQ], F32)  # x16 biases, split-hd order
    bks = dparam("bks", [P, 2 * NQ], F32)
    b1 = dparam("b1", [P, FC], F32)
    bvb = dparam("bvb", [P, D], F32)
    b2b = dparam("b2b", [P, D], F32)       # natural scale
    g1b = dparam("g1b", [P, D], F32)
    be1b = dparam("be1b", [P, D], F32)
    g2b = dparam("g2b", [P, D], F32)
    be2b = dparam("be2b", [P, D], F32)
    vones = dparam("vones", [P, KT])       # value 16 (denominator row)
    hc8 = dparam("hc8", [P, 2 * QT])       # F1 bias rows: [16,0...;0...]

    y = nc.dram_tensor("y", [QT, D], F32, kind="ExternalOutput")

    with tile.TileContext(nc) as tc, ExitStack() as es:
        # Pre-load the one activation table that serves every ACT func we
        # use (Exp for softmax, Ln+Exp for the LN rsqrt): without this the
        # first-fit chooser thrashes exp<->ln tables at 1283ns per load.
        from concourse.hw_specs import get_activation_tables
        _tabs = get_activation_tables(nc.m.arch)
        _idx = next(i for i, (_, s) in enumerate(_tabs.items())
                    if AF.Exp in s and AF.Ln in s)
        nc.scalar.add_instruction(mybir.InstLoadActFuncSet(
            name=nc.scalar.bass.get_next_instruction_name(),
            act_func_set_id=_idx, ins=[], outs=[]))

        pers = es.enter_context(tc.tile_pool(name="pers", bufs=1))
        ident = pers.tile([P, P], F32, tag="ident")
        make_identity(nc, ident[:])
        bqs_t = pers.tile([P, 2 * NQ], F32, tag="bqs")
        bks_t = pers.tile([P, 2 * NQ], F32, tag="bks")
        b1_t = pers.tile([P, FC], F32, tag="b1")
        bvb_t = pers.tile([P, D], F32, tag="bvb")
        # b2b reuses bvb's slot: bvb is dead after V-proj, long before
        # the LNT chunks fold b2 into the residual.
        b2b_t = pers.tile([P, D], F32, tag="bvb", name="b2b_t")

        xT_t = pers.tile([P, DC, L], F8, tag="xT")
        xTq_t = pers.tile([P, DC, QT], F8, tag="xTq")
        v_aug = pers.tile([P, KT, H * (HD + 1)], F8, tag="vaug")
        ones_t = pers.tile([P, KT], F8, tag="ones")
        qT_all = pers.tile([P, NQ, 2, QT], F8, tag="qT")
        kT_all = pers.tile([P, NQ, 2, L], F8, tag="kT")
        outSB = pers.tile([P, NPAIR, QT], F8, tag="outSB")
        h_t = pers.tile([P, TT, D], F32, tag="h")
        hT_t = pers.tile([P, DC + 2, QT], F8, tag="hT")
        ff1 = pers.tile([P, FC, QT], F8, tag="ff1")
        xq_s = pers.tile([P, TT, D], F32, tag="xqs")
        wqs_t = pers.tile([P, 2 * NQ, DC2, 2 * P], F8, tag="wqs")
        wks_t = pers.tile([P, 2 * NQ, DC2, 2 * P], F8, tag="wks")
        wv_t = pers.tile([P, 2, DC2, 2 * 512], F8, tag="wv")
        wo_t = pers.tile([P, NP2, 2 * D], F8, tag="wof")
        w2_t = pers.tile([P, FC2 // 2, 2, 2 * D], F8, tag="w2")
        if affine:
            g1b_t = pers.tile([P, D], F32, tag="g1b")
            be1b_t = pers.tile([P, D], F32, tag="be1b")
            g2b_t = pers.tile([P, D], F32, tag="g2b")
            be2b_t = pers.tile([P, D], F32, tag="be2b")
        else:
            g1b_t = be1b_t = g2b_t = be2b_t = None

        # startup DMAs, ordered for earliest first-exp: the DMA engine pool
        # is serially occupied, so issue exactly what unblocks Q/K/V first.
        nc.scalar.dma_start(bqs_t[:], bqs[:])
        nc.scalar.dma_start(bks_t[:], bks[:])
        nc.sync.dma_start(xTq_t[:], xTq8.rearrange("(c p) t -> p c t", p=P))
        wqs_r = wqs.rearrange("p b (c m) -> p b c m", c=DC2)
        wks_r = wks.rearrange("p b (c m) -> p b c m", c=DC2)
        nc.sync.dma_start(wqs_t[:, 0:2], wqs_r[:, 0:2])
        nc.sync.dma_start(wks_t[:, 0:2], wks_r[:, 0:2])
        for blk in range(4):
            nc.sync.dma_start(
                xT_t[:, :, blk * 512:(blk + 1) * 512],
                xT8.rearrange("(c p) t -> p c t", p=P)[
                    :, :, blk * 512:(blk + 1) * 512])
        wv_r = wv8.rearrange("p (v c m) -> p v c m", v=2, c=DC2)
        nc.sync.dma_start(wv_t[:, 0:1], wv_r[:, 0:1])
        nc.sync.dma_start(wqs_t[:, 2:8], wqs_r[:, 2:8])
        nc.sync.dma_start(wks_t[:, 2:8], wks_r[:, 2:8])
        nc.sync.dma_start(wv_t[:, 1:2], wv_r[:, 1:2])
        nc.scalar.dma_start(b1_t[:], b1[:])
        nc.scalar.dma_start(bvb_t[:], bvb[:])
        nc.scalar.dma_start(ones_t[:], vones[:])
        nc.scalar.dma_start(
            hT_t[:, DC:DC + 2, :],
            hc8.rearrange("p (k t) -> p k t", k=2))

        nc.vector.tensor_copy(
            v_aug[:].rearrange("p t (h c) -> p t h c", c=HD + 1)[:, :, :, HD],
            ones_t[:, :, None].to_broadcast([P, KT, H]))
        # mid-kernel loads, all needed only by the post-attention chunks
        nc.sync.dma_start(xq_s[:], xq.rearrange("(t p) d -> p t d", p=P))
        nc.sync.dma_start(wo_t[:], wo8.rearrange("p (j m) -> p j m", j=NP2))
        nc.sync.dma_start(w2_t[:], w28.rearrange("p s (g m) -> p s g m", g=2))
        if affine:
            nc.scalar.dma_start(g1b_t[:], g1b[:])
            nc.scalar.dma_start(be1b_t[:], be1b[:])
            nc.scalar.dma_start(g2b_t[:], g2b[:])
            nc.scalar.dma_start(be2b_t[:], be2b[:])

        stp = es.enter_context(tc.tile_pool(name="stp", bufs=2, space="PSUM"))
        pvp = es.enter_context(tc.tile_pool(name="pvp", bufs=2, space="PSUM"))
        ppp = es.enter_context(tc.tile_pool(name="ppp", bufs=3))
        atd = es.enter_context(tc.tile_pool(name="atd", bufs=2))
        lnp = es.enter_context(tc.tile_pool(name="lnp", bufs=2))
        fp = es.enter_context(tc.tile_pool(name="fp", bufs=2))
        ft = es.enter_context(tc.tile_pool(name="ft", bufs=2))
        kvp_es = ExitStack()
        kvp = kvp_es.enter_context(tc.tile_pool(name="kvp", bufs=2,
                                                space="PSUM"))

        def emit_qproj(g):
            for i in range(2):
                b = 2 * g + i
                ps = kvp.tile([P, 512], F32, tag="kv", name=f"qps_{b}")
                for d2 in range(DC2):
                    nc.tensor.matmul(
                        ps[:],
                        wqs_t[:, b, d2, :].rearrange("p (i m) -> p i m", i=2),
                        xTq_t[:, 2 * d2:2 * d2 + 2, :],
                        start=(d2 == 0), stop=(d2 == DC2 - 1), perf_mode=DR)
                nc.vector.tensor_scalar(
                    qT_all[:, g, i, :], ps[:],
                    bqs_t[:, b:b + 1], None, ALU.add)

        def emit_kproj(g, i, blks=range(4)):
            """kT_all[:, g, i, :] for one hd-half of head-quad g."""
            b = 2 * g + i
            for blk in blks:         # 512-key blocks
                ps = kvp.tile([P, 512], F32, tag="kv",
                              name=f"kps_{g}_{i}_{blk}")
                for d2 in range(DC2):
                    nc.tensor.matmul(
                        ps[:],
                        wks_t[:, b, d2, :].rearrange("p (i m) -> p i m", i=2),
                        xT_t[:, 2 * d2:2 * d2 + 2,
                             blk * 512:(blk + 1) * 512],
                        start=(d2 == 0), stop=(d2 == DC2 - 1), perf_mode=DR)
                nc.vector.tensor_scalar(
                    kT_all[:, g, i, blk * 512:(blk + 1) * 512], ps[:],
                    bks_t[:, b:b + 1], None, ALU.add)

        def emit_vsub(vh, sub):
            """v_aug columns for v-half vh, key tiles 4*sub..4*sub+3."""
            for tt in range(4 * sub, 4 * sub + 4):
                ps = kvp.tile([P, 512], F32, tag="kv",
                              name=f"vps_{vh}_{tt}")
                for d2 in range(DC2):
                    nc.tensor.matmul(
                        ps[:],
                        xT_t[:, 2 * d2:2 * d2 + 2, tt * P:(tt + 1) * P],
                        wv_t[:, vh, d2, :].rearrange("p (i n) -> p i n", i=2),
                        start=(d2 == 0), stop=(d2 == DC2 - 1), perf_mode=DR)
                dst = v_aug[:, tt, :].rearrange(
                    "p (h c) -> p h c", c=HD + 1)[:, vh * 8:(vh + 1) * 8, 0:HD]
                nc.vector.tensor_tensor(
                    dst, ps[:].rearrange("p (h c) -> p h c", c=HD),
                    bvb_t[:, vh * 512:(vh + 1) * 512].rearrange(
                        "p (h c) -> p h c", c=HD),
                    ALU.add)

        emit_qproj(0)
        for blk in range(4):
            emit_kproj(0, 0, blks=(blk,))
            emit_kproj(0, 1, blks=(blk,))
        emit_vsub(0, 0)

        fillers = []

        def drain():
            if fillers:
                fillers.pop(0)()

        def emit_attn(qh, pr):
            g, j0 = pr // 2, 2 * (pr % 2)
            pvs = [pvp.tile([P, 512], F32, tag="pv", name=f"pv_{qh}_{pr}_{h2}")
                   for h2 in range(2)]
            for grp in range(KT // 2):
                st = stp.tile([P, 2, 2, 256], F32, tag="st",
                              name=f"st_{qh}_{pr}_{grp}")
                for h2 in range(2):
                    j = j0 + h2
                    rows = slice(32 * j, 32 * j + 32)
                    for k in range(2):
                        kt = 2 * grp + k
                        nc.tensor.matmul(
                            st[:, h2, k, :],
                            kT_all[rows, g, :, kt * P:(kt + 1) * P],
                            qT_all[rows, g, :, qh * QH:(qh + 1) * QH],
                            start=True, stop=True, perf_mode=DR,
                            tile_position=(32 * j, 0))
                pp = ppp.tile([P, 2, 2, 256], F8, tag="pp",
                              name=f"pp_{qh}_{pr}_{grp}")
                nc.scalar.activation(pp[:], st[:], AF.Exp,
                                     scale=0.125 / (WS * WS))
                for h2 in range(2):
                    h = 2 * pr + h2
                    nc.tensor.matmul(
                        pvs[h2][:HD + 1, :QH],
                        v_aug[:, 2 * grp:2 * grp + 2,
                              h * (HD + 1):(h + 1) * (HD + 1)],
                        pp[:, h2, :, :],
                        start=(grp == 0), stop=(grp == KT // 2 - 1),
                        perf_mode=DR)
                if grp in (0, 2, 4):
                    drain()
            for h2 in range(2):
                rows = slice(h2 * HD, h2 * HD + HD)
                # bounce PV to SBUF so the PSUM bank frees after one short
                # DVE copy instead of the whole recip/bcast/normalize chain
                # (the bank gates the next pair's PV accumulator).
                pvc = atd.tile([P, QH], F32, tag="pvc",
                               name=f"pvc_{qh}_{pr}_{h2}")
                nc.vector.tensor_copy(pvc[:HD + 1, :], pvs[h2][:HD + 1, :QH])
                # reciprocal in place over the denominator row of pvc
                nc.vector.reciprocal(pvc[HD:HD + 1, :], pvc[HD:HD + 1, :])
                denb = atd.tile([HD, QH], F32, tag="denb",
                                name=f"denb_{qh}_{pr}_{h2}")
                nc.gpsimd.partition_broadcast(denb[:], pvc[HD:HD + 1, :])
                nc.gpsimd.tensor_tensor(
                    outSB[rows, pr, qh * QH:(qh + 1) * QH],
                    pvc[:HD, :], denb[:], ALU.mult)

        # ---------- post-attention chunk emitters (token-tile tg) ----------
        postp_es = ExitStack()
        postp = [None]

        def emit_O(qh, tt):
            tg = 2 * qh + tt
            for oc in range(2):
                ps = postp[0].tile([P, 512], F32, tag="post",
                                   name=f"ops_{tg}_{oc}")
                for j2 in range(NP2):
                    nc.tensor.matmul(
                        ps[:],
                        outSB[:, 2 * j2:2 * j2 + 2, tg * P:(tg + 1) * P],
                        wo_t[:, j2, :].rearrange(
                            "p (i o) -> p i o", i=2)[:, :, oc * 512:(oc + 1) * 512],
                        start=(j2 == 0), stop=(j2 == NP2 - 1), perf_mode=DR)
                nc.vector.tensor_tensor(
                    h_t[:, tg, oc * 512:(oc + 1) * 512], ps[:],
                    xq_s[:, tg, oc * 512:(oc + 1) * 512], ALU.add)

        def emit_LNT(qh, tt):
            tg = 2 * qh + tt
            _layernorm(nc, lnp, h_t[:, tg, :], h_t[:, tg, :],
                       g1b_t, be1b_t, affine)
            for dcg in range(2):
                tp = postp[0].tile([P, 512], F32, tag="post",
                                   name=f"tp_{tg}_{dcg}")
                for k in range(4):
                    dc = 4 * dcg + k
                    nc.tensor.transpose(
                        tp[:, k * P:(k + 1) * P],
                        h_t[:, tg, dc * P:(dc + 1) * P], ident[:])
                nc.vector.tensor_copy(
                    hT_t[:, 4 * dcg:4 * dcg + 4, tg * P:(tg + 1) * P],
                    tp[:].rearrange("p (k m) -> p k m", k=4))
            # fold the fc2 bias into the residual (after transposes read h)
            nc.gpsimd.tensor_tensor(h_t[:, tg, :], h_t[:, tg, :],
                                    b2b_t[:], ALU.add)

        def emit_F1(qh, fcg, use_act=False, pre=None, split_tt=False):
            if pre is not None:
                w1_t = pre
            else:
                w1_t = fp.tile([P, 4, DC2 + 1, 2 * P], F8, tag="w1s")
                nc.sync.dma_start(w1_t[:], w18[:, fcg, :].rearrange(
                    "p (f c m) -> p f c m", f=4, c=DC2 + 1))
            tparts = ((0, 1) if split_tt else (None,))
            for u in range(2):
                ps = postp[0].tile([P, 512], F32, tag="post",
                                   name=f"f1_{qh}_{fcg}_{u}")
                for tl in tparts:
                    cols = slice(qh * QH, (qh + 1) * QH) if tl is None else \
                        slice((2 * qh + tl) * P, (2 * qh + tl + 1) * P)
                    w = QH if tl is None else P
                    for f in (2 * u, 2 * u + 1):
                        fl = f - 2 * u
                        base = fl * QH + (0 if tl is None else tl * P)
                        for d2 in range(DC2 + 1):
                            nc.tensor.matmul(
                                ps[:, base:base + w],
                                w1_t[:, f, d2, :].rearrange(
                                    "p (i m) -> p i m", i=2),
                                hT_t[:, 2 * d2:2 * d2 + 2, cols],
                                start=(d2 == 0), stop=(d2 == DC2),
                                perf_mode=DR, skip_group_check=True)
                    dst = ff1[:, 4 * fcg + 2 * u:4 * fcg + 2 * u + 2, cols]
                    src_ap = ps[:].rearrange("p (f n) -> p f n", f=2) \
                        if tl is None else \
                        ps[:].rearrange("p (f n) -> p f n", f=2)[
                            :, :, tl * P:(tl + 1) * P]
                    if use_act and u == 1:
                        nc.scalar.activation(dst, src_ap, AF.Relu)
                    else:
                        nc.vector.tensor_scalar(dst, src_ap, 0.0, None,
                                                ALU.max)

        def emit_F2(qh, tt, oc, part=2):
            """part: 0 = first half of the ff contraction, 1 = second half
            (+LN2/store), 2 = whole thing in one chunk."""
            tg = 2 * qh + tt
            if oc == 0 and part != 1:
                _f2_t2[tg] = ft.tile([P, D], F32, tag="t2", name=f"t2_{tg}")
            lo = FC2 // 2 if part == 1 else 0
            hi = FC2 // 2 if part == 0 else FC2
            ps = postp[0].tile([P, 512], F32, tag="post",
                               name=f"f2_{tg}_{oc}_{part}")
            for f2 in range(lo, hi):
                s, g2 = f2 // 2, f2 % 2
                nc.tensor.matmul(
                    ps[:],
                    ff1[:, 2 * f2:2 * f2 + 2, tg * P:(tg + 1) * P],
                    w2_t[:, s, g2, :].rearrange(
                        "p (i o) -> p i o", i=2)[:, :, oc * 512:(oc + 1) * 512],
                    start=(f2 == lo), stop=(f2 == hi - 1), perf_mode=DR)
            t2 = _f2_t2[tg]
            acc = h_t[:, tg, oc * 512:(oc + 1) * 512] if part != 1 \
                else t2[:, oc * 512:(oc + 1) * 512]
            nc.vector.scalar_tensor_tensor(
                t2[:, oc * 512:(oc + 1) * 512], ps[:], 1.0 / (WS * WS),
                acc, ALU.mult, ALU.add)
            if oc == 1 and part != 0:
                _layernorm(nc, lnp, t2[:], t2[:], g2b_t, be2b_t, affine)
                nc.sync.dma_start(
                    y.rearrange("(t p) d -> p t d", p=P)[:, tg, :], t2[:])

        _f2_t2 = {}

        # ---------------- schedule ----------------
        fillers.extend([
            lambda: emit_vsub(0, 1), lambda: emit_vsub(0, 2),
            lambda: emit_vsub(0, 3),
            lambda: emit_qproj(1),
            lambda: emit_kproj(1, 0), lambda: emit_kproj(1, 1),
            lambda: emit_vsub(1, 0), lambda: emit_vsub(1, 1),
            lambda: emit_qproj(2),
            lambda: emit_kproj(2, 0), lambda: emit_kproj(2, 1),
            lambda: emit_vsub(1, 2), lambda: emit_vsub(1, 3),
            lambda: emit_qproj(3),
            lambda: emit_kproj(3, 0), lambda: emit_kproj(3, 1),
        ])
        for pr in range(NPAIR):
            emit_attn(0, pr)
        while fillers:
            drain()
        kvp_es.close()
        nc.scalar.dma_start(b2b_t[:], b2b[:])
        postp[0] = postp_es.enter_context(
            tc.tile_pool(name="postp", bufs=2, space="PSUM"))
        fillers.extend(
            [lambda tt=tt: emit_O(0, tt) for tt in range(2)] +
            [lambda tt=tt: emit_LNT(0, tt) for tt in range(2)] +
            [lambda f=f: emit_F1(0, f) for f in range(FC // 4)] +
            [lambda tt=tt, oc=oc: emit_F2(0, tt, oc)
             for tt in range(2) for oc in range(2)])
        for pr in range(NPAIR):
            emit_attn(1, pr)
        while fillers:
            drain()
        w1_pre = {}
        for f in range(2):
            w1_pre[f] = fp.tile([P, 4, DC2 + 1, 2 * P], F8, tag="w1s",
                                name=f"w1pre_{f}")
            nc.sync.dma_start(w1_pre[f][:], w18[:, f, :].rearrange(
                "p (f c m) -> p f c m", f=4, c=DC2 + 1))
        for tt in range(2):
            emit_O(1, tt)
            emit_LNT(1, tt)
        for f in range(FC // 4):
            emit_F1(1, f, use_act=True, pre=w1_pre.get(f))
            if f == 4:
                for tt in range(2):
                    for oc in range(2):
                        emit_F2(1, tt, oc, part=0)
        for tt in range(2):
            for oc in range(2):
                emit_F2(1, tt, oc, part=1)
        postp_es.close()

    nc.compile()
    return nc


def _layernorm(nc, pool, dst, src, g_t, be_t, affine):
    """dst = (src - mean)/sqrt(var + eps) [* g + be], row-wise over 1024.

    bn_stats/bn_aggr produce mean+var in one DVE pass. rsqrt is computed
    as exp(-0.5*ln(v)) on ACT (both funcs live in one activation table,
    so no table thrash with the attention exps) and refined with one
    Newton step on DVE.
    """
    stats = pool.tile([P, 2, 6], F32, tag="ln_st")
    nc.vector.bn_stats(stats[:, 0, :], src[:, 0:D // 2])
    nc.vector.bn_stats(stats[:, 1, :], src[:, D // 2:D])
    mv = pool.tile([P, 2], F32, tag="ln_mv")
    nc.vector.bn_aggr(mv[:], stats[:])
    vv = pool.tile([P, 1], F32, tag="ln_v")
    nc.vector.tensor_scalar(vv[:], mv[:, 1:2], EPS, None, ALU.add)
    lnv = pool.tile([P, 1], F32, tag="ln_ln")
    nc.scalar.activation(lnv[:], vv[:], AF.Ln)
    r = pool.tile([P, 1], F32, tag="ln_r")
    nc.scalar.activation(r[:], lnv[:], AF.Exp, scale=-0.5)
    # one Newton step: r <- r * (1.5 - 0.5 * vv * r^2)
    t = pool.tile([P, 1], F32, tag="ln_t")
    nc.vector.tensor_tensor(t[:], r[:], r[:], ALU.mult)
    nc.vector.tensor_tensor(t[:], t[:], vv[:], ALU.mult)
    nc.vector.tensor_scalar(t[:], t[:], -0.5, 1.5, ALU.mult, ALU.add)
    nc.vector.tensor_tensor(r[:], r[:], t[:], ALU.mult)
    nc.vector.tensor_scalar(dst, src, mv[:, 0:1], r[:], ALU.subtract, ALU.mult)
    if affine:
        nc.vector.tensor_tensor(dst, dst, g_t[:], ALU.mult)
        nc.vector.tensor_tensor(dst, dst, be_t[:], ALU.add)


def _hc8():
    h = np.zeros((P, 2 * QT), np.float32)
    h[0, :QT] = WS
    return h.astype(mybir.dt.np(F8))


def make_in_maps(x, w_qkv, b_qkv, w_o, b_o, g1, be1, w1, b1, w2, b2, g2, be2):
    f = np.float32
    f8 = mybir.dt.np(F8)
    x = np.asarray(x, f)
    w_qkv = np.asarray(w_qkv, f)
    b_qkv = np.asarray(b_qkv, f)
    bc = lambda v: np.ascontiguousarray(
        np.broadcast_to(np.asarray(v, f).reshape(1, D), (P, D)))

    # [d, h*64+hd] -> [p][(g i)][d2 ik m] with m=32j+r -> (head 4g+j, hd 32i+r)
    def qk_split(w):
        t = (w * WS).reshape(DC2, 2, P, NQ, 4, 2, 32)
        t = t.transpose(2, 3, 5, 0, 1, 4, 6)    # [p, g, i, d2, ik, j, r]
        return np.ascontiguousarray(t.reshape(P, 2 * NQ, 2 * DC2 * P)).astype(f8)

    def bias_split(b):
        t = (b * WS).reshape(NQ, 4, 2, 32).transpose(1, 3, 0, 2)  # [j, r, g, i]
        return np.ascontiguousarray(t.reshape(P, 2 * NQ))

    wv_h = np.ascontiguousarray(
        (w_qkv[:, 2 * D:] * WS).reshape(DC2, 2, P, 2, 512)
        .transpose(2, 3, 0, 1, 4).reshape(P, 2 * DC2 * 2 * 512)).astype(f8)
    wo_h = np.ascontiguousarray(
        np.asarray(w_o, f).reshape(NP2, 2, P, D).transpose(2, 0, 1, 3)
        .reshape(P, NP2 * 2 * D)).astype(f8)
    w1_base = ((np.asarray(w1, f) * WS).reshape(DC2, 2, P, FC // 4, 4, P)
               .transpose(2, 3, 4, 0, 1, 5))          # [p, f4, f, d2, ik, m]
    w1_bias = np.zeros((P, FC // 4, 4, 1, 2, P), f)
    w1_bias[0, :, :, 0, 0, :] = np.asarray(b1, f).reshape(FC // 4, 4, P)
    w1_h = np.ascontiguousarray(
        np.concatenate([w1_base, w1_bias], axis=3)
        .reshape(P, FC // 4, 4 * (DC2 + 1) * 2 * P)).astype(f8)
    w2_h = np.ascontiguousarray(
        (np.asarray(w2, f) * WS).reshape(FC2 // 2, 2, 2, P, D)
        .transpose(3, 0, 1, 2, 4).reshape(P, FC2 // 2, 2 * 2 * D)).astype(f8)

    shared = {
        "wqs": qk_split(w_qkv[:, :D]),
        "wks": qk_split(w_qkv[:, D:2 * D]),
        "wv8": wv_h, "wo8": wo_h, "w18": w1_h, "w28": w2_h,
        "bqs": bias_split(b_qkv[:D]),
        "bks": bias_split(b_qkv[D:2 * D]),
        "b1": np.ascontiguousarray((np.asarray(b1, f) * WS).reshape(FC, P).T),
        "bvb": bc(np.asarray(b_qkv[2 * D:], f) * WS), "b2b": bc(b2),
        "g1b": bc(g1), "be1b": bc(be1), "g2b": bc(g2), "be2b": bc(be2),
        "vones": np.full((P, KT), WS, f).astype(f8),
        "hc8": _hc8(),
    }
    in_maps = []
    for c in range(8):
        n, qi = divmod(c, 4)
        xT8n = np.ascontiguousarray(x[n].T).astype(f8)
        m = dict(shared)
        m["xT8"] = xT8n
        m["xTq8"] = np.ascontiguousarray(xT8n[:, qi * QT:(qi + 1) * QT])
        m["xq"] = np.ascontiguousarray(x[n, qi * QT:(qi + 1) * QT, :]
                                       + np.asarray(b_o, f).reshape(1, D))
        in_maps.append(m)
    return in_maps


def get_nc(affine=True):
    if affine not in _CACHED_NC:
        _CACHED_NC[affine] = _build_nc(affine)
    return _CACHED_NC[affine]


def kernel(**inputs):
    in_maps = make_in_maps(**inputs)
    affine = not (np.all(np.asarray(inputs["g1"]) == 1)
                  and np.all(np.asarray(inputs["be1"]) == 0)
                  and np.all(np.asarray(inputs["g2"]) == 1)
                  and np.all(np.asarray(inputs["be2"]) == 0))
    nc = get_nc(affine)
    # The axon-proxied NRT occasionally reports a transient
    # NRT_EXEC_UNIT_UNRECOVERABLE on a cold first dispatch; a plain retry
    # has always succeeded with bit-identical results, so recover inline.
    last_err = None
    for _ in range(3):
        try:
            res = run_bass_kernel_spmd(nc, in_maps, list(range(8))).results
            break
        except Exception as e:  # noqa: BLE001
            last_err = e
    else:
        raise last_err
    yout = np.empty((NB, L, D), np.float32)
    for c in range(8):
        n, qi = divmod(c, 4)
        yout[n, qi * QT:(qi + 1) * QT] = res[c]["y"]
    return yout


if __name__ == "__main__":
    rng = np.random.default_rng(0)
    demo = {
        "x": rng.standard_normal((NB, L, D)).astype(np.float32),
        "w_qkv": rng.standard_normal((D, 3 * D)).astype(np.float32) * 0.03,
        "b_qkv": rng.standard_normal(3 * D).astype(np.float32) * 0.03,
        "w_o": rng.standard_normal((D, D)).astype(np.float32) * 0.03,
        "b_o": rng.standard_normal(D).astype(np.float32) * 0.03,
        "g1": np.ones(D, np.float32), "be1": np.zeros(D, np.float32),
        "w1": rng.standard_normal((D, FF)).astype(np.float32) * 0.03,
        "b1": rng.standard_normal(FF).astype(np.float32) * 0.03,
        "w2": rng.standard_normal((FF, D)).astype(np.float32) * 0.015,
        "b2": rng.standard_normal(D).astype(np.float32) * 0.015,
        "g2": np.ones(D, np.float32), "be2": np.zeros(D, np.float32),
    }
    out = kernel(**demo)
    print("kernel output:", out.shape, out.dtype, np.abs(out).mean())


# revision 27
# speedup vs baseline: 1.1731x; 1.0352x over previous
"""Trainium2 Bass kernel for a transformer encoder layer (nn_Encoder).

x:[2,2048,1024] f32. 8 NeuronCores, data-parallel: core c handles batch
n=c//4, query rows qi=c%4 (512 tokens). K/V are recomputed per core for
the full batch (x4 redundancy) to avoid collectives (~300us for the
8.4MB all-reduce this would replace).

All matmuls are fp8 e4m3 DoubleRow (2x PE column rate, 256-deep
contraction per instruction). K/Q are produced in a split-hd layout
([32 partitions, 2 k-tiles] per head) so even the hd=64 score matmuls
run DoubleRow. Weights are host-scaled x16 into the fp8 normal range;
the scale unwinds via the softmax ones-row (=16), the exp scale
(0.125/256) and a /256 on the fc2 PSUM. Residuals/LayerNorm stay f32.

The softmax exp stream on the Activation engine (~133us) is the
critical resource. Attention is split into two query-halves: while
half 1's exps run, half 0's output-proj/LN1/FFN execute in the PE/DVE
shadow (emitted as interleaved filler chunks); K/V projections for
later head-quads fill the shadow of half 0.
"""
import os
import sys
from contextlib import ExitStack

for _p in ("/opt/trn_rl_repo", "/root/.axon_site/_ro/trn_rl_repo"):
    if os.path.isdir(_p) and _p not in sys.path:
        sys.path.insert(0, _p)

import numpy as np
import concourse.bass as bass
import concourse.mybir as mybir
import concourse.tile as tile
from concourse import bacc
from concourse.bass_utils import run_bass_kernel_spmd
from concourse.masks import make_identity

F32 = mybir.dt.float32
F8 = mybir.dt.float8e4
AF = mybir.ActivationFunctionType
ALU = mybir.AluOpType
DR = mybir.MatmulPerfMode.DoubleRow

D = 1024
H = 16
HD = 64
FF = 4096
L = 2048
NB = 2
P = 128
QT = 512          # query tokens per core
QH = QT // 2      # query half
DC = D // P       # 8 chunks of the model dim
DC2 = DC // 2     # 4 DoubleRow chunk-pairs
KT = L // P       # 16 key tiles
FC = FF // P      # 32 ff chunks
FC2 = FC // 2     # 16 ff chunk-pairs
TT = QT // P      # 4 own-token tiles
NPAIR = H // 2    # 8 head pairs
NP2 = NPAIR // 2  # 4 pair-pairs
NQ = H // 4       # 4 head quads (scores split layout)
EPS = 1e-5
WS = 16.0         # host weight scale (fp8 range)

_CACHED_NC = {}


def _build_nc(affine=True):
    nc = bacc.Bacc("TRN2", target_bir_lowering=False)

    def dparam(name, shape, dt=F8):
        return nc.dram_tensor(name, shape, dt, kind="ExternalInput")

    xT8 = dparam("xT8", [D, L])            # x[n].T, fp8
    xTq8 = dparam("xTq8", [D, QT])         # own-token columns of xT, fp8
    xq = dparam("xq", [QT, D], F32)        # own tokens, natural (residual)
    # weights: partition-major fp8, DoubleRow k-tile-pair layouts
    wqs = dparam("wqs", [P, 2 * NQ, 2 * DC2 * P])    # [p][(g i)][d2 ik m]
    wks = dparam("wks", [P, 2 * NQ, 2 * DC2 * P])
    wv8 = dparam("wv8", [P, 2 * DC2 * 2 * 512])      # [p][vh d2 ik n]
    wo8 = dparam("wo8", [P, NP2 * 2 * D])            # [p][j ik o]
    w18 = dparam("w18", [P, FC // 4, 4 * (DC2 + 1) * 2 * P])  # [p][f4][f d2 ik m]
    w28 = dparam("w28", [P, FC2 // 2, 2 * 2 * D])        # [p][s][g ik o]
    bqs = dparam("bqs", [P, 2 * NQ], F32)  # x16 biases, split-hd order
    bks = dparam("bks", [P, 2 * NQ], F32)
    b1 = dparam("b1", [P, FC], F32)
    bvb = dparam("bvb", [P, D], F32)
    b2b = dparam("b2b", [P, D], F32)       # natural scale
    g1b = dparam("g1b", [P, D], F32)
    be1b = dparam("be1b", [P, D], F32)
    g2b = dparam("g2b", [P, D], F32)
    be2b = dparam("be2b", [P, D], F32)
    vones = dparam("vones", [P, KT])       # value 16 (denominator row)
    hc8 = dparam("hc8", [P, 2 * QT])       # F1 bias rows: [16,0...;0...]

    y = nc.dram_tensor("y", [QT, D], F32, kind="ExternalOutput")

    with tile.TileContext(nc) as tc, ExitStack() as es:
        # Pre-load the one activation table that serves every ACT func we
        # use (Exp for softmax, Ln+Exp for the LN rsqrt): without this the
        # first-fit chooser thrashes exp<->ln tables at 1283ns per load.
        from concourse.hw_specs import get_activation_tables
        _tabs = get_activation_tables(nc.m.arch)
        _idx = next(i for i, (_, s) in enumerate(_tabs.items())
                    if AF.Exp in s and AF.Ln in s)
        nc.scalar.add_instruction(mybir.InstLoadActFuncSet(
            name=nc.scalar.bass.get_next_instruction_name(),
            act_func_set_id=_idx, ins=[], outs=[]))

        pers = es.enter_context(tc.tile_pool(name="pers", bufs=1))
        ident = pers.tile([P, P], F32, tag="ident")
        make_identity(nc, ident[:])
        bqs_t = pers.tile([P, 2 * NQ], F32, tag="bqs")
        bks_t = pers.tile([P, 2 * NQ], F32, tag="bks")
        b1_t = pers.tile([P, FC], F32, tag="b1")
        bvb_t = pers.tile([P, D], F32, tag="bvb")
        # b2b reuses bvb's slot: bvb is dead after V-proj, long before
        # the LNT chunks fold b2 into the residual.
        b2b_t = pers.tile([P, D], F32, tag="bvb", name="b2b_t")

        xT_t = pers.tile([P, DC, L], F8, tag="xT")
        xTq_t = pers.tile([P, DC, QT], F8, tag="xTq")
        v_aug = pers.tile([P, KT, H * (HD + 1)], F8, tag="vaug")
        ones_t = pers.tile([P, KT], F8, tag="ones")
        qT_all = pers.tile([P, NQ, 2, QT], F8, tag="qT")
        kT_all = pers.tile([P, NQ, 2, L], F8, tag="kT")
        outSB = pers.tile([P, NPAIR, QT], F8, tag="outSB")
        h_t = pers.tile([P, TT, D], F32, tag="h")
        hT_t = pers.tile([P, DC + 2, QT], F8, tag="hT")
        ff1 = pers.tile([P, FC, QT], F8, tag="ff1")
        xq_s = pers.tile([P, TT, D], F32, tag="xqs")
        wqs_t = pers.tile([P, 2 * NQ, DC2, 2 * P], F8, tag="wqs")
        wks_t = pers.tile([P, 2 * NQ, DC2, 2 * P], F8, tag="wks")
        wv_t = pers.tile([P, 2, DC2, 2 * 512], F8, tag="wv")
        wo_t = pers.tile([P, NP2, 2 * D], F8, tag="wof")
        w2_t = pers.tile([P, FC2 // 2, 2, 2 * D], F8, tag="w2")
        if affine:
            g1b_t = pers.tile([P, D], F32, tag="g1b")
            be1b_t = pers.tile([P, D], F32, tag="be1b")
            g2b_t = pers.tile([P, D], F32, tag="g2b")
            be2b_t = pers.tile([P, D], F32, tag="be2b")
        else:
            g1b_t = be1b_t = g2b_t = be2b_t = None

        # startup DMAs, ordered for earliest first-exp: the DMA engine pool
        # is serially occupied, so issue exactly what unblocks Q/K/V first.
        nc.scalar.dma_start(bqs_t[:], bqs[:])
        nc.scalar.dma_start(bks_t[:], bks[:])
        nc.sync.dma_start(xTq_t[:], xTq8.rearrange("(c p) t -> p c t", p=P))
        wqs_r = wqs.rearrange("p b (c m) -> p b c m", c=DC2)
        wks_r = wks.rearrange("p b (c m) -> p b c m", c=DC2)
        nc.sync.dma_start(wqs_t[:, 0:2], wqs_r[:, 0:2])
        nc.sync.dma_start(wks_t[:, 0:2], wks_r[:, 0:2])
        for blk in range(4):
            nc.sync.dma_start(
                xT_t[:, :, blk * 512:(blk + 1) * 512],
                xT8.rearrange("(c p) t -> p c t", p=P)[
                    :, :, blk * 512:(blk + 1) * 512])
        wv_r = wv8.rearrange("p (v c m) -> p v c m", v=2, c=DC2)
        nc.sync.dma_start(wv_t[:, 0:1], wv_r[:, 0:1])
        nc.sync.dma_start(wqs_t[:, 2:8], wqs_r[:, 2:8])
        nc.sync.dma_start(wks_t[:, 2:8], wks_r[:, 2:8])
        nc.sync.dma_start(wv_t[:, 1:2], wv_r[:, 1:2])
        nc.scalar.dma_start(b1_t[:], b1[:])
        nc.scalar.dma_start(bvb_t[:], bvb[:])
        nc.scalar.dma_start(ones_t[:], vones[:])
        nc.scalar.dma_start(
            hT_t[:, DC:DC + 2, :],
            hc8.rearrange("p (k t) -> p k t", k=2))

        nc.vector.tensor_copy(
            v_aug[:].rearrange("p t (h c) -> p t h c", c=HD + 1)[:, :, :, HD],
            ones_t[:, :, None].to_broadcast([P, KT, H]))
        # mid-kernel loads, all needed only by the post-attention chunks
        nc.sync.dma_start(xq_s[:], xq.rearrange("(t p) d -> p t d", p=P))
        nc.sync.dma_start(wo_t[:], wo8.rearrange("p (j m) -> p j m", j=NP2))
        nc.sync.dma_start(w2_t[:], w28.rearrange("p s (g m) -> p s g m", g=2))
        if affine:
            nc.scalar.dma_start(g1b_t[:], g1b[:])
            nc.scalar.dma_start(be1b_t[:], be1b[:])
            nc.scalar.dma_start(g2b_t[:], g2b[:])
            nc.scalar.dma_start(be2b_t[:], be2b[:])

        ppp = es.enter_context(tc.tile_pool(name="ppp", bufs=3))
        atd = es.enter_context(tc.tile_pool(name="atd", bufs=2))
        lnp = es.enter_context(tc.tile_pool(name="lnp", bufs=2))
        fp = es.enter_context(tc.tile_pool(name="fp", bufs=2))
        ft = es.enter_context(tc.tile_pool(name="ft", bufs=2))
        attn_es = ExitStack()
        stp = attn_es.enter_context(tc.tile_pool(name="stp", bufs=2,
                                                 space="PSUM"))
        pvp = attn_es.enter_context(tc.tile_pool(name="pvp", bufs=2,
                                                 space="PSUM"))
        kvp_es = ExitStack()
        kvp = kvp_es.enter_context(tc.tile_pool(name="kvp", bufs=2,
                                                space="PSUM"))

        def emit_qproj(g):
            for i in range(2):
                b = 2 * g + i
                ps = kvp.tile([P, 512], F32, tag="kv", name=f"qps_{b}")
                for d2 in range(DC2):
                    nc.tensor.matmul(
                        ps[:],
                        wqs_t[:, b, d2, :].rearrange("p (i m) -> p i m", i=2),
                        xTq_t[:, 2 * d2:2 * d2 + 2, :],
                        start=(d2 == 0), stop=(d2 == DC2 - 1), perf_mode=DR)
                nc.vector.tensor_scalar(
                    qT_all[:, g, i, :], ps[:],
                    bqs_t[:, b:b + 1], None, ALU.add)

        def emit_kproj(g, i, blks=range(4)):
            """kT_all[:, g, i, :] for one hd-half of head-quad g."""
            b = 2 * g + i
            for blk in blks:         # 512-key blocks
                ps = kvp.tile([P, 512], F32, tag="kv",
                              name=f"kps_{g}_{i}_{blk}")
                for d2 in range(DC2):
                    nc.tensor.matmul(
                        ps[:],
                        wks_t[:, b, d2, :].rearrange("p (i m) -> p i m", i=2),
                        xT_t[:, 2 * d2:2 * d2 + 2,
                             blk * 512:(blk + 1) * 512],
                        start=(d2 == 0), stop=(d2 == DC2 - 1), perf_mode=DR)
                nc.vector.tensor_scalar(
                    kT_all[:, g, i, blk * 512:(blk + 1) * 512], ps[:],
                    bks_t[:, b:b + 1], None, ALU.add)

        def emit_vsub(vh, sub):
            """v_aug columns for v-half vh, key tiles 4*sub..4*sub+3."""
            for tt in range(4 * sub, 4 * sub + 4):
                ps = kvp.tile([P, 512], F32, tag="kv",
                              name=f"vps_{vh}_{tt}")
                for d2 in range(DC2):
                    nc.tensor.matmul(
                        ps[:],
                        xT_t[:, 2 * d2:2 * d2 + 2, tt * P:(tt + 1) * P],
                        wv_t[:, vh, d2, :].rearrange("p (i n) -> p i n", i=2),
                        start=(d2 == 0), stop=(d2 == DC2 - 1), perf_mode=DR)
                dst = v_aug[:, tt, :].rearrange(
                    "p (h c) -> p h c", c=HD + 1)[:, vh * 8:(vh + 1) * 8, 0:HD]
                nc.vector.tensor_tensor(
                    dst, ps[:].rearrange("p (h c) -> p h c", c=HD),
                    bvb_t[:, vh * 512:(vh + 1) * 512].rearrange(
                        "p (h c) -> p h c", c=HD),
                    ALU.add)

        emit_qproj(0)
        for blk in range(4):
            emit_kproj(0, 0, blks=(blk,))
            emit_kproj(0, 1, blks=(blk,))
        emit_vsub(0, 0)

        fillers = []

        def drain():
            if fillers:
                fillers.pop(0)()

        def emit_attn(qh, pr):
            g, j0 = pr // 2, 2 * (pr % 2)
            pvs = [pvp.tile([P, 512], F32, tag="pv", name=f"pv_{qh}_{pr}_{h2}")
                   for h2 in range(2)]
            for grp in range(KT // 2):
                st = stp.tile([P, 2, 2, 256], F32, tag="st",
                              name=f"st_{qh}_{pr}_{grp}")
                for h2 in range(2):
                    j = j0 + h2
                    rows = slice(32 * j, 32 * j + 32)
                    for k in range(2):
                        kt = 2 * grp + k
                        nc.tensor.matmul(
                            st[:, h2, k, :],
                            kT_all[rows, g, :, kt * P:(kt + 1) * P],
                            qT_all[rows, g, :, qh * QH:(qh + 1) * QH],
                            start=True, stop=True, perf_mode=DR,
                            tile_position=(32 * j, 0))
                pp = ppp.tile([P, 2, 2, 256], F8, tag="pp",
                              name=f"pp_{qh}_{pr}_{grp}")
                nc.scalar.activation(pp[:], st[:], AF.Exp,
                                     scale=0.125 / (WS * WS))
                for h2 in range(2):
                    h = 2 * pr + h2
                    nc.tensor.matmul(
                        pvs[h2][:HD + 1, :QH],
                        v_aug[:, 2 * grp:2 * grp + 2,
                              h * (HD + 1):(h + 1) * (HD + 1)],
                        pp[:, h2, :, :],
                        start=(grp == 0), stop=(grp == KT // 2 - 1),
                        perf_mode=DR)
                if grp in (0, 2, 4):
                    drain()
            for h2 in range(2):
                rows = slice(h2 * HD, h2 * HD + HD)
                # bounce PV to SBUF so the PSUM bank frees after one short
                # DVE copy instead of the whole recip/bcast/normalize chain
                # (the bank gates the next pair's PV accumulator).
                pvc = atd.tile([P, QH], F32, tag="pvc",
                               name=f"pvc_{qh}_{pr}_{h2}")
                nc.vector.tensor_copy(pvc[:HD + 1, :], pvs[h2][:HD + 1, :QH])
                # reciprocal in place over the denominator row of pvc
                nc.vector.reciprocal(pvc[HD:HD + 1, :], pvc[HD:HD + 1, :])
                denb = atd.tile([HD, QH], F32, tag="denb",
                                name=f"denb_{qh}_{pr}_{h2}")
                nc.gpsimd.partition_broadcast(denb[:], pvc[HD:HD + 1, :])
                nc.gpsimd.tensor_tensor(
                    outSB[rows, pr, qh * QH:(qh + 1) * QH],
                    pvc[:HD, :], denb[:], ALU.mult)

        # ---------- post-attention chunk emitters (token-tile tg) ----------
        postp_es = ExitStack()
        postp = [None]

        def emit_O(qh, tt):
            tg = 2 * qh + tt
            for oc in range(2):
                ps = postp[0].tile([P, 512], F32, tag="post",
                                   name=f"ops_{tg}_{oc}")
                for j2 in range(NP2):
                    nc.tensor.matmul(
                        ps[:],
                        outSB[:, 2 * j2:2 * j2 + 2, tg * P:(tg + 1) * P],
                        wo_t[:, j2, :].rearrange(
                            "p (i o) -> p i o", i=2)[:, :, oc * 512:(oc + 1) * 512],
                        start=(j2 == 0), stop=(j2 == NP2 - 1), perf_mode=DR)
                nc.vector.tensor_tensor(
                    h_t[:, tg, oc * 512:(oc + 1) * 512], ps[:],
                    xq_s[:, tg, oc * 512:(oc + 1) * 512], ALU.add)

        def emit_LNT(qh, tt):
            tg = 2 * qh + tt
            _layernorm(nc, lnp, h_t[:, tg, :], h_t[:, tg, :],
                       g1b_t, be1b_t, affine)
            for dcg in range(2):
                tp = postp[0].tile([P, 512], F32, tag="post",
                                   name=f"tp_{tg}_{dcg}")
                for k in range(4):
                    dc = 4 * dcg + k
                    nc.tensor.transpose(
                        tp[:, k * P:(k + 1) * P],
                        h_t[:, tg, dc * P:(dc + 1) * P], ident[:])
                nc.vector.tensor_copy(
                    hT_t[:, 4 * dcg:4 * dcg + 4, tg * P:(tg + 1) * P],
                    tp[:].rearrange("p (k m) -> p k m", k=4))
            # fold the fc2 bias into the residual (after transposes read h)
            nc.gpsimd.tensor_tensor(h_t[:, tg, :], h_t[:, tg, :],
                                    b2b_t[:], ALU.add)

        def emit_F1(qh, fcg, use_act=False, pre=None, split_tt=False):
            if pre is not None:
                w1_t = pre if not hasattr(pre, "tile") else pre
            else:
                w1_t = fp.tile([P, 4, DC2 + 1, 2 * P], F8, tag="w1s")
                nc.sync.dma_start(w1_t[:], w18[:, fcg, :].rearrange(
                    "p (f c m) -> p f c m", f=4, c=DC2 + 1))
            tparts = ((0, 1) if split_tt else (None,))
            for u in range(2):
                ps = postp[0].tile([P, 512], F32, tag="post",
                                   name=f"f1_{qh}_{fcg}_{u}")
                for tl in tparts:
                    cols = slice(qh * QH, (qh + 1) * QH) if tl is None else \
                        slice((2 * qh + tl) * P, (2 * qh + tl + 1) * P)
                    w = QH if tl is None else P
                    for f in (2 * u, 2 * u + 1):
                        fl = f - 2 * u
                        base = fl * QH + (0 if tl is None else tl * P)
                        for d2 in range(DC2 + 1):
                            nc.tensor.matmul(
                                ps[:, base:base + w],
                                w1_t[:, f, d2, :].rearrange(
                                    "p (i m) -> p i m", i=2),
                                hT_t[:, 2 * d2:2 * d2 + 2, cols],
                                start=(d2 == 0), stop=(d2 == DC2),
                                perf_mode=DR, skip_group_check=True)
                    dst = ff1[:, 4 * fcg + 2 * u:4 * fcg + 2 * u + 2, cols]
                    src_ap = ps[:].rearrange("p (f n) -> p f n", f=2) \
                        if tl is None else \
                        ps[:].rearrange("p (f n) -> p f n", f=2)[
                            :, :, tl * P:(tl + 1) * P]
                    if use_act and u == 1:
                        nc.scalar.activation(dst, src_ap, AF.Relu)
                    else:
                        nc.vector.tensor_scalar(dst, src_ap, 0.0, None,
                                                ALU.max)

        def emit_F2(qh, tt, oc, part=2):
            """part: 0 = first half of the ff contraction, 1 = second half
            (+LN2/store), 2 = whole thing in one chunk."""
            tg = 2 * qh + tt
            if oc == 0 and part != 1:
                _f2_t2[tg] = ft.tile([P, D], F32, tag="t2", name=f"t2_{tg}")
            lo = FC2 // 2 if part == 1 else 0
            hi = FC2 // 2 if part == 0 else FC2
            ps = postp[0].tile([P, 512], F32, tag="post",
                               name=f"f2_{tg}_{oc}_{part}")
            for f2 in range(lo, hi):
                s, g2 = f2 // 2, f2 % 2
                nc.tensor.matmul(
                    ps[:],
                    ff1[:, 2 * f2:2 * f2 + 2, tg * P:(tg + 1) * P],
                    w2_t[:, s, g2, :].rearrange(
                        "p (i o) -> p i o", i=2)[:, :, oc * 512:(oc + 1) * 512],
                    start=(f2 == lo), stop=(f2 == hi - 1), perf_mode=DR)
            t2 = _f2_t2[tg]
            acc = h_t[:, tg, oc * 512:(oc + 1) * 512] if part != 1 \
                else t2[:, oc * 512:(oc + 1) * 512]
            nc.vector.scalar_tensor_tensor(
                t2[:, oc * 512:(oc + 1) * 512], ps[:], 1.0 / (WS * WS),
                acc, ALU.mult, ALU.add)
            if oc == 1 and part != 0:
                _layernorm(nc, lnp, t2[:], t2[:], g2b_t, be2b_t, affine)
                nc.sync.dma_start(
                    y.rearrange("(t p) d -> p t d", p=P)[:, tg, :], t2[:])

        _f2_t2 = {}

        # ---------------- schedule ----------------
        fillers.extend([
            lambda: emit_vsub(0, 1), lambda: emit_vsub(0, 2),
            lambda: emit_vsub(0, 3),
            lambda: emit_qproj(1),
            lambda: emit_kproj(1, 0), lambda: emit_kproj(1, 1),
            lambda: emit_vsub(1, 0), lambda: emit_vsub(1, 1),
            lambda: emit_qproj(2),
            lambda: emit_kproj(2, 0), lambda: emit_kproj(2, 1),
            lambda: emit_vsub(1, 2), lambda: emit_vsub(1, 3),
            lambda: emit_qproj(3),
            lambda: emit_kproj(3, 0), lambda: emit_kproj(3, 1),
        ])
        for pr in range(NPAIR):
            emit_attn(0, pr)
        while fillers:
            drain()
        kvp_es.close()
        nc.scalar.dma_start(b2b_t[:], b2b[:])
        postp[0] = postp_es.enter_context(
            tc.tile_pool(name="postp", bufs=2, space="PSUM"))
        fillers.extend(
            [lambda tt=tt: emit_O(0, tt) for tt in range(2)] +
            [lambda tt=tt: emit_LNT(0, tt) for tt in range(2)] +
            [lambda f=f: emit_F1(0, f) for f in range(FC // 4)] +
            [lambda tt=tt, oc=oc: emit_F2(0, tt, oc)
             for tt in range(2) for oc in range(2)])
        # tail w1 head-chunks ride the dead xT slot; DMA them while the
        # qh1 exps still run (the DMA engines are idle then)
        w1c_t = pers.tile([P, 3, 4 * (DC2 + 1) * 2 * P], F8, tag="xT",
                          name="w1c_t")
        for f in range(3):
            nc.sync.dma_start(w1c_t[:, f, :], w18[:, f, :])
        for pr in range(NPAIR):
            emit_attn(1, pr)
        while fillers:
            drain()
        postp_es.close()
        attn_es.close()
        tail_es = ExitStack()
        postp[0] = tail_es.enter_context(
            tc.tile_pool(name="tailp", bufs=6, space="PSUM"))
        w1_pre = {}
        for f in range(3):
            w1_pre[f] = w1c_t[:, f, :].rearrange(
                "p (f c m) -> p f c m", f=4, c=DC2 + 1)
        for f in range(3, 5):
            w1_pre[f] = fp.tile([P, 4, DC2 + 1, 2 * P], F8, tag="w1s",
                                name=f"w1pre_{f}")
            nc.sync.dma_start(w1_pre[f][:], w18[:, f, :].rearrange(
                "p (f c m) -> p f c m", f=4, c=DC2 + 1))
        for tt in range(2):
            emit_O(1, tt)
            emit_LNT(1, tt)
        for f in range(FC // 4):
            emit_F1(1, f, use_act=True, pre=w1_pre.get(f))
            if f == 4:
                for tt in range(2):
                    for oc in range(2):
                        emit_F2(1, tt, oc, part=0)
        for tt in range(2):
            for oc in range(2):
                emit_F2(1, tt, oc, part=1)
        tail_es.close()

    nc.compile()
    return nc


def _layernorm(nc, pool, dst, src, g_t, be_t, affine):
    """dst = (src - mean)/sqrt(var + eps) [* g + be], row-wise over 1024.

    bn_stats/bn_aggr produce mean+var in one DVE pass. rsqrt is computed
    as exp(-0.5*ln(v)) on ACT (both funcs live in one activation table,
    so no table thrash with the attention exps) and refined with one
    Newton step on DVE.
    """
    stats = pool.tile([P, 2, 6], F32, tag="ln_st")
    nc.vector.bn_stats(stats[:, 0, :], src[:, 0:D // 2])
    nc.vector.bn_stats(stats[:, 1, :], src[:, D // 2:D])
    mv = pool.tile([P, 2], F32, tag="ln_mv")
    nc.vector.bn_aggr(mv[:], stats[:])
    vv = pool.tile([P, 1], F32, tag="ln_v")
    nc.vector.tensor_scalar(vv[:], mv[:, 1:2], EPS, None, ALU.add)
    lnv = pool.tile([P, 1], F32, tag="ln_ln")
    nc.scalar.activation(lnv[:], vv[:], AF.Ln)
    r = pool.tile([P, 1], F32, tag="ln_r")
    nc.scalar.activation(r[:], lnv[:], AF.Exp, scale=-0.5)
    # one Newton step: r <- r * (1.5 - 0.5 * vv * r^2)
    t = pool.tile([P, 1], F32, tag="ln_t")
    nc.vector.tensor_tensor(t[:], r[:], r[:], ALU.mult)
    nc.vector.tensor_tensor(t[:], t[:], vv[:], ALU.mult)
    nc.vector.tensor_scalar(t[:], t[:], -0.5, 1.5, ALU.mult, ALU.add)
    nc.vector.tensor_tensor(r[:], r[:], t[:], ALU.mult)
    nc.vector.tensor_scalar(dst, src, mv[:, 0:1], r[:], ALU.subtract, ALU.mult)
    if affine:
        nc.vector.tensor_tensor(dst, dst, g_t[:], ALU.mult)
        nc.vector.tensor_tensor(dst, dst, be_t[:], ALU.add)


def _hc8():
    h = np.zeros((P, 2 * QT), np.float32)
    h[0, :QT] = WS
    return h.astype(mybir.dt.np(F8))


def make_in_maps(x, w_qkv, b_qkv, w_o, b_o, g1, be1, w1, b1, w2, b2, g2, be2):
    f = np.float32
    f8 = mybir.dt.np(F8)
    x = np.asarray(x, f)
    w_qkv = np.asarray(w_qkv, f)
    b_qkv = np.asarray(b_qkv, f)
    bc = lambda v: np.ascontiguousarray(
        np.broadcast_to(np.asarray(v, f).reshape(1, D), (P, D)))

    # [d, h*64+hd] -> [p][(g i)][d2 ik m] with m=32j+r -> (head 4g+j, hd 32i+r)
    def qk_split(w):
        t = (w * WS).reshape(DC2, 2, P, NQ, 4, 2, 32)
        t = t.transpose(2, 3, 5, 0, 1, 4, 6)    # [p, g, i, d2, ik, j, r]
        return np.ascontiguousarray(t.reshape(P, 2 * NQ, 2 * DC2 * P)).astype(f8)

    def bias_split(b):
        t = (b * WS).reshape(NQ, 4, 2, 32).transpose(1, 3, 0, 2)  # [j, r, g, i]
        return np.ascontiguousarray(t.reshape(P, 2 * NQ))

    wv_h = np.ascontiguousarray(
        (w_qkv[:, 2 * D:] * WS).reshape(DC2, 2, P, 2, 512)
        .transpose(2, 3, 0, 1, 4).reshape(P, 2 * DC2 * 2 * 512)).astype(f8)
    wo_h = np.ascontiguousarray(
        np.asarray(w_o, f).reshape(NP2, 2, P, D).transpose(2, 0, 1, 3)
        .reshape(P, NP2 * 2 * D)).astype(f8)
    w1_base = ((np.asarray(w1, f) * WS).reshape(DC2, 2, P, FC // 4, 4, P)
               .transpose(2, 3, 4, 0, 1, 5))          # [p, f4, f, d2, ik, m]
    w1_bias = np.zeros((P, FC // 4, 4, 1, 2, P), f)
    w1_bias[0, :, :, 0, 0, :] = np.asarray(b1, f).reshape(FC // 4, 4, P)
    w1_h = np.ascontiguousarray(
        np.concatenate([w1_base, w1_bias], axis=3)
        .reshape(P, FC // 4, 4 * (DC2 + 1) * 2 * P)).astype(f8)
    w2_h = np.ascontiguousarray(
        (np.asarray(w2, f) * WS).reshape(FC2 // 2, 2, 2, P, D)
        .transpose(3, 0, 1, 2, 4).reshape(P, FC2 // 2, 2 * 2 * D)).astype(f8)

    shared = {
        "wqs": qk_split(w_qkv[:, :D]),
        "wks": qk_split(w_qkv[:, D:2 * D]),
        "wv8": wv_h, "wo8": wo_h, "w18": w1_h, "w28": w2_h,
        "bqs": bias_split(b_qkv[:D]),
        "bks": bias_split(b_qkv[D:2 * D]),
        "b1": np.ascontiguousarray((np.asarray(b1, f) * WS).reshape(FC, P).T),
        "bvb": bc(np.asarray(b_qkv[2 * D:], f) * WS), "b2b": bc(b2),
        "g1b": bc(g1), "be1b": bc(be1), "g2b": bc(g2), "be2b": bc(be2),
        "vones": np.full((P, KT), WS, f).astype(f8),
        "hc8": _hc8(),
    }
    in_maps = []
    for c in range(8):
        n, qi = divmod(c, 4)
        xT8n = np.ascontiguousarray(x[n].T).astype(f8)
        m = dict(shared)
        m["xT8"] = xT8n
        m["xTq8"] = np.ascontiguousarray(xT8n[:, qi * QT:(qi + 1) * QT])
        m["xq"] = np.ascontiguousarray(x[n, qi * QT:(qi + 1) * QT, :]
                                       + np.asarray(b_o, f).reshape(1, D))
        in_maps.append(m)
    return in_maps


def get_nc(affine=True):
    if affine not in _CACHED_NC:
        _CACHED_NC[affine] = _build_nc(affine)
    return _CACHED_NC[affine]


def kernel(**inputs):
    in_maps = make_in_maps(**inputs)
    affine = not (np.all(np.asarray(inputs["g1"]) == 1)
                  and np.all(np.asarray(inputs["be1"]) == 0)
                  and np.all(np.asarray(inputs["g2"]) == 1)
                  and np.all(np.asarray(inputs["be2"]) == 0))
    nc = get_nc(affine)
    # The axon-proxied NRT occasionally reports a transient
    # NRT_EXEC_UNIT_UNRECOVERABLE on a cold first dispatch; a plain retry
    # has always succeeded with bit-identical results, so recover inline.
    last_err = None
    for _ in range(3):
        try:
            res = run_bass_kernel_spmd(nc, in_maps, list(range(8))).results
            break
        except Exception as e:  # noqa: BLE001
            last_err = e
    else:
        raise last_err
    yout = np.empty((NB, L, D), np.float32)
    for c in range(8):
        n, qi = divmod(c, 4)
        yout[n, qi * QT:(qi + 1) * QT] = res[c]["y"]
    return yout


if __name__ == "__main__":
    rng = np.random.default_rng(0)
    demo = {
        "x": rng.standard_normal((NB, L, D)).astype(np.float32),
        "w_qkv": rng.standard_normal((D, 3 * D)).astype(np.float32) * 0.03,
        "b_qkv": rng.standard_normal(3 * D).astype(np.float32) * 0.03,
        "w_o": rng.standard_normal((D, D)).astype(np.float32) * 0.03,
        "b_o": rng.standard_normal(D).astype(np.float32) * 0.03,
        "g1": np.ones(D, np.float32), "be1": np.zeros(D, np.float32),
        "w1": rng.standard_normal((D, FF)).astype(np.float32) * 0.03,
        "b1": rng.standard_normal(FF).astype(np.float32) * 0.03,
        "w2": rng.standard_normal((FF, D)).astype(np.float32) * 0.015,
        "b2": rng.standard_normal(D).astype(np.float32) * 0.015,
        "g2": np.ones(D, np.float32), "be2": np.zeros(D, np.float32),
    }
    out = kernel(**demo)
    print("kernel output:", out.shape, out.dtype, np.abs(out).mean())
